# revision 34
# baseline (speedup 1.0000x reference)
"""Trainium2 Bass kernel for nn_CrossAttention (B=4, N=2048, E=768, H=8).

Sharding: 8 cores = 4 batches x 2 head-groups (4 heads of 96 dims each).
Each core computes its batch's attention for its 4 heads plus the partial
output projection; the host sums the two head-group partials per batch and
adds bo.

Design (v4, ~255us vs the 312us v0 baseline):
  - matmul stationaries are 128 columns where free (K/Q chunks, p tiles)
    for the compiler-automatic Fast Weight Load path; V uses exact 97-wide
    head blocks (96 dims + the softmax-rowsum ones column injected via the
    bias add) -- padding V cost more projection columns than FWL saved.
  - K/Q projections run 128-row packed (3 tiles of 128 dims, not 4 heads
    of 96) and SBUF->SBUF DMAs on the idle software-DGE ring repack heads
    1-3 to partition base 0; head 0 aliases the packed tile directly.
  - the scalar engine runs ONLY exp; the O^T copy and normalization run on
    DVE (reciprocal_approx_accurate over all 97 partitions -- custom DVE
    ops break at nonzero partition base), and the rowsum reciprocal row is
    broadcast with a 1-row bf16 matmul contracting on partition 96 via
    tile_position=(96,0).  No partition-shift DMA in the norm chain.
  - single software-pipelined emission: projection chunks, repacks and the
    qc0 out-projection are "filler units" drained two per kv-iteration
    into the attention loop so the PE never idles (pstate protection);
    qc1's out-projection is split h0+h1 early / h2+h3 tail, with the
    h0+h1 partials aliased onto xkv's dead SBUF storage.
  - PSUM is exactly 8 banks: tag s 2x[128,1024]f32, po 1x[128,1024],
    x 2x[128,512]; the tail ping-pongs over the then-dead s/po tags.
  - 55 dummy matmuls on a memset scratch tile keep the PE pstate warm
    through the ~20us initial DMA wait (the ramp resets on idle gaps).
  - DMA: ~115GB/s per ring, ~1us fixed cost per DMA, and Act-engine
    dispatches block the engine while its ring is busy -- so the scalar
    ring gets only the two early xq waves, gpsimd (software DGE) carries
    the weights + repacks, sync the rest; qc0 output stores rotate over
    all three rings, tail stores use the two hardware rings only.
"""

import os
import sys
import types
from collections import deque

import numpy as np

# ---------------------------------------------------------------------------
# NTFF profile hook (the agent image's antenv lacks axon_hooks; degrade OK)
# ---------------------------------------------------------------------------
def _install_ntff_hook():
    if "antenv.axon_hooks" in sys.modules:
        return
    try:
        hooks = types.ModuleType("antenv.axon_hooks")
        hooks._hook = None
        hooks.set_axon_ntff_profile_hook = lambda h: setattr(hooks, "_hook", h)
        hooks.get_axon_ntff_profile_hook = lambda: hooks._hook
        sys.modules["antenv.axon_hooks"] = hooks
        import antenv

        antenv.axon_hooks = hooks
        from trn_agent_boot.trn_boot import _ntff_profile_via_ctypes

        so = "/opt/axon/libaxon_pjrt.so"
        if os.path.exists(so):
            hooks.set_axon_ntff_profile_hook(_ntff_profile_via_ctypes(so))
    except Exception:
        pass


_install_ntff_hook()

import concourse.bacc as bacc
import concourse.tile as tile
import concourse.mybir as mybir
from concourse import bass_utils
from concourse.alu_op_type import AluOpType

F32 = mybir.dt.float32
F32R = mybir.dt.float32r
BF16 = mybir.dt.bfloat16

B = 4
NQ = 2048
NKV = 2048
E = 768
H_LOCAL = 4  # heads per core
HD = 96  # head dim
HP = 128  # padded head dim (FWL wants 128-wide stationaries)
D = H_LOCAL * HD  # 384 local proj dim
DP = H_LOCAL * HP  # 512 padded local proj dim
VW = HD + 1  # 97: per-head V block (96 dims + rowsum ones column)
DV = H_LOCAL * VW  # 388
ET = E // 128  # 6 contraction tiles
KV_T = NKV // 128  # 16 kv tiles
QT_T = NQ // 128  # 16 q tiles
INV_SQRT_E = 1.0 / float(np.sqrt(np.float32(E)))


def build_nc():
    nc = bacc.Bacc("TRN2", target_bir_lowering=False, debug=False)

    xq_t = nc.dram_tensor("xq_t", [E, NQ], BF16, kind="ExternalInput")
    xkv_t = nc.dram_tensor("xkv_t", [E, NKV], BF16, kind="ExternalInput")
    wq_t = nc.dram_tensor("wq_t", [E, D], BF16, kind="ExternalInput")
    wk_t = nc.dram_tensor("wk_t", [E, D], BF16, kind="ExternalInput")
    wv_t = nc.dram_tensor("wv_t", [E, DV], BF16, kind="ExternalInput")
    wo_t = nc.dram_tensor("wo_t", [D, E], BF16, kind="ExternalInput")
    consts_t = nc.dram_tensor("consts_t", [128, 6 + DV], F32, kind="ExternalInput")
    out = nc.dram_tensor("out", [NQ, E], F32, kind="ExternalOutput")

    with tile.TileContext(nc) as tc:
        with (
            nc.allow_low_precision(reason="bf16 matmuls and f32r broadcast"),
            tc.tile_pool(name="persist", bufs=1) as persist,
            tc.tile_pool(name="psum", bufs=1, space="PSUM") as pp,
            tc.tile_pool(name="sb", bufs=1) as sb,
        ):
            # ---------------- persistent SBUF tensors ----------------
            KT = persist.tile([HD, H_LOCAL, NKV], BF16)  # K^T per head
            QT = persist.tile([HD, H_LOCAL, NQ], BF16)  # Q^T per head
            # V: [kv-token, kv-tile, head-block(96 v dims + ones + 31 zeros)]
            V = persist.tile([128, KV_T, DV], BF16)
            attn = persist.tile([HD, H_LOCAL, NQ], BF16)  # normalized attn^T
            wo_sb = persist.tile([HD, H_LOCAL, E], BF16)
            wq_sb = persist.tile([128, ET, D], BF16)
            wk_sb = persist.tile([128, ET, D], BF16)
            wv_sb = persist.tile([128, ET, DV], BF16)
            K3 = persist.tile([128, 3, NKV], BF16)  # packed K^T staging
            Q3 = persist.tile([128, 3, NQ], BF16)  # packed Q^T staging
            consts_sb = persist.tile([128, 6 + DV], F32)
            bk_sb = consts_sb[:, 0:3]
            bq_sb = consts_sb[:, 3:6]
            bv_sb = consts_sb[:, 6 : 6 + DV]
            ones_sb = persist.tile([HD + 1, HP], BF16)  # row 96 = [1]*96+[0]*32
            xkv_sb = persist.tile([128, ET, NKV], BF16)
            # qc1 outproj h0+h1 partials alias xkv's storage (dead by then):
            # [128, 6, 2048] bf16 == [128, 6144] f32/2 == [128, 8, 768] f32
            ob01 = xkv_sb[:].bitcast(F32).rearrange("p t n -> p (t n)").rearrange(
                "p (a b) -> p a b", a=8
            )
            xq_sb = persist.tile([128, ET, NQ], BF16)

            # ---------------- ACT table preload + warm tile ----------------
            warm = persist.tile([128, 512], BF16)
            warm_f = persist.tile([1, 8], F32)
            nc.vector.memset(warm[:], 0.0)
            nc.vector.memset(warm_f[:], 0.0)
            nc.scalar.activation(
                warm_f[:], warm_f[:], mybir.ActivationFunctionType.Exp,
                scale=1.0,
            )

            # ---------------- input DMAs ----------------
            # Act-engine DMA dispatches block the engine while its ring is
            # busy, so the scalar ring gets ONLY the two early xq waves (ACT
            # must be free for exp by ~15us). gpsimd(software DGE) carries
            # the weights, sync everything else.
            nc.gpsimd.dma_start(
                wq_sb[:], wq_t[:].rearrange("(t p) n -> p t n", p=128)
            )
            nc.gpsimd.dma_start(
                wk_sb[:], wk_t[:].rearrange("(t p) n -> p t n", p=128)
            )
            nc.gpsimd.dma_start(
                wv_sb[:], wv_t[:].rearrange("(t p) n -> p t n", p=128)
            )
            nc.scalar.dma_start(
                xq_sb[:, 0:3, 0:1024],
                xq_t[0:384, 0:1024].rearrange("(t p) n -> p t n", p=128),
            )
            nc.scalar.dma_start(
                xq_sb[:, 3:6, 0:1024],
                xq_t[384:768, 0:1024].rearrange("(t p) n -> p t n", p=128),
            )
            nc.sync.dma_start(
                xkv_sb[:, 0:3, 0:1024],
                xkv_t[0:384, 0:1024].rearrange("(t p) n -> p t n", p=128),
            )
            nc.sync.dma_start(
                xkv_sb[:, 3:6, 0:1024],
                xkv_t[384:768, 0:1024].rearrange("(t p) n -> p t n", p=128),
            )
            nc.sync.dma_start(consts_sb[:], consts_t[:])
            nc.sync.dma_start(
                xkv_sb[:, :, 1024:2048],
                xkv_t[:, 1024:2048].rearrange("(t p) n -> p t n", p=128),
            )
            nc.sync.dma_start(
                xq_sb[:, :, 1024:2048],
                xq_t[:, 1024:2048].rearrange("(t p) n -> p t n", p=128),
            )
            nc.sync.dma_start(
                wo_sb[:], wo_t[:].rearrange("(h p) n -> p h n", p=HD)
            )
            nc.vector.memset(ones_sb[HD : HD + 1, 0:HD], 1.0)
            nc.vector.memset(ones_sb[HD : HD + 1, HD:HP], 0.0)

            # PE warm-up dummies (see preamble): the pstate ramp resets on
            # idle gaps; dummies span the ~20us initial DMA wait.

            # ---------------- psum slot helpers ----------------
            # tags: s 2x[128,1024], po 1x[128,1024], x 2x[128,512]: 8 banks
            def psum_s():
                return pp.tile([128, 1024], F32, tag="s", bufs=2, name="ps_s")

            def psum_po():
                return pp.tile([128, 1024], F32, tag="po", bufs=1, name="ps_po")

            def psum_x():
                return pp.tile([128, 512], F32, tag="x", bufs=2, name="ps_x")

            # ---------------- projection / outproj unit emitters ----------
            def kp_unit(w_sb, b3, dst3, x_sb, t3, c):
                # dst3[:, t3, c*512:(c+1)*512] = w[t3-block] @ x^T + b
                ps = psum_x()
                for e in range(ET):
                    nc.tensor.matmul(
                        ps[:],
                        w_sb[:, e, t3 * 128 : (t3 + 1) * 128],
                        x_sb[:, e, c * 512 : (c + 1) * 512],
                        start=(e == 0),
                        stop=(e == ET - 1),
                    )
                nc.vector.tensor_scalar_add(
                    out=dst3[:, t3, c * 512 : (c + 1) * 512],
                    in0=ps[:],
                    scalar1=b3[:, t3 : t3 + 1],
                )

            def repack_unit(src3, dst, ring, h, c):
                ring = nc.gpsimd
                # head h dims (96h..96h+95) -> dst[:, h, c-chunk], base 0
                cs = slice(c * 512, (c + 1) * 512)
                lo = h * HD
                t_lo, r_lo = divmod(lo, 128)
                n0 = min(128 - r_lo, HD)
                ring.dma_start(
                    dst[0:n0, h, cs], src3[r_lo : r_lo + n0, t_lo, cs]
                )
                if n0 < HD:
                    ring.dma_start(
                        dst[n0:HD, h, cs],
                        src3[0 : HD - n0, t_lo + 1, cs],
                    )

            def v_unit(t):
                # V[:, t, :] = x_t @ Wv^T + bve  (bve carries the ones column)
                ps = psum_x()
                for e in range(ET):
                    nc.tensor.matmul(
                        ps[:, 0:DV],
                        xkv_sb[:, e, t * 128 : (t + 1) * 128],
                        wv_sb[:, e, :],
                        start=(e == 0),
                        stop=(e == ET - 1),
                    )
                nc.vector.tensor_tensor(
                    out=V[:, t, :],
                    in0=ps[:, 0:DV],
                    in1=bv_sb[:],
                    op=AluOpType.add,
                )

            ob_i = [0]

            def op_unit(qt, fa, fb):
                # out[qt] = attn^T_qt.T @ Wo^T  (accumulate 4 heads)
                for h in range(H_LOCAL):
                    nc.tensor.matmul(
                        fa,
                        attn[:, h, qt * 128 : (qt + 1) * 128],
                        wo_sb[:, h, 0:512],
                        start=(h == 0),
                        stop=(h == H_LOCAL - 1),
                    )
                for h in range(H_LOCAL):
                    nc.tensor.matmul(
                        fb,
                        attn[:, h, qt * 128 : (qt + 1) * 128],
                        wo_sb[:, h, 512:768],
                        start=(h == 0),
                        stop=(h == H_LOCAL - 1),
                    )
                ob = sb.tile([128, E], F32, tag="ob", bufs=2, name="ob")
                nc.vector.tensor_copy(ob[:, 0:512], fa)
                nc.vector.tensor_copy(ob[:, 512:768], fb)
                ring = (nc.sync, nc.scalar, nc.gpsimd)[ob_i[0] % 3]
                ob_i[0] += 1
                ring.dma_start(out[qt * 128 : (qt + 1) * 128, :], ob[:])

            def op_unit_x(qt):
                fa = psum_x()
                fb = psum_x()
                op_unit(qt, fa[:], fb[:, 0:256])

            def op_h01_unit(qt):
                # heads 0-1 accumulation for qc1's out tiles
                fa = psum_x()
                fb = psum_x()
                for h in range(2):
                    nc.tensor.matmul(
                        fa[:],
                        attn[:, h, qt * 128 : (qt + 1) * 128],
                        wo_sb[:, h, 0:512],
                        start=(h == 0),
                        stop=(h == 1),
                    )
                    nc.tensor.matmul(
                        fb[:, 0:256],
                        attn[:, h, qt * 128 : (qt + 1) * 128],
                        wo_sb[:, h, 512:768],
                        start=(h == 0),
                        stop=(h == 1),
                    )
                nc.vector.tensor_copy(ob01[:, qt - 8, 0:512], fa[:])
                nc.vector.tensor_copy(ob01[:, qt - 8, 512:768], fb[:, 0:256])

            def op_h23_unit(qt, fa, fb):
                for h in range(2, 4):
                    nc.tensor.matmul(
                        fa,
                        attn[:, h, qt * 128 : (qt + 1) * 128],
                        wo_sb[:, h, 0:512],
                        start=(h == 2),
                        stop=(h == 3),
                    )
                    nc.tensor.matmul(
                        fb,
                        attn[:, h, qt * 128 : (qt + 1) * 128],
                        wo_sb[:, h, 512:768],
                        start=(h == 2),
                        stop=(h == 3),
                    )
                ob = sb.tile([128, E], F32, tag="ob", bufs=2, name="ob")
                nc.vector.tensor_tensor(
                    out=ob[:, 0:512], in0=fa, in1=ob01[:, qt - 8, 0:512],
                    op=AluOpType.add,
                )
                nc.vector.tensor_tensor(
                    out=ob[:, 512:768], in0=fb, in1=ob01[:, qt - 8, 512:768],
                    op=AluOpType.add,
                )
                ring = (nc.sync, nc.scalar, nc.gpsimd)[ob_i[0] % 3]
                ob_i[0] += 1
                ring.dma_start(out[qt * 128 : (qt + 1) * 128, :], ob[:])

            # ---------------- filler machinery ----------------
            fillerA = deque()  # projection units: (deadline_block, fn)
            fillerB = deque()  # outproj units for qc0 (unlocked at block 5)
            fillerC = deque()  # qc1 outproj h0+h1 partials (unlocked blk 7)
            b_unlocked = [False]
            c_unlocked = [False]

            def drain_one():
                if fillerA:
                    fillerA.popleft()[1]()
                elif b_unlocked[0] and fillerB:
                    fillerB.popleft()()
                elif c_unlocked[0] and fillerC:
                    fillerC.popleft()()

            def drain_deadline(blk):
                while fillerA and fillerA[0][0] <= blk:
                    fillerA.popleft()[1]()

            # ---------------- attention block ----------------
            def norm_rest_for(o_sb, rs, h, qc):
                def norm_rest():
                    for n in range(2):
                        bcT = psum_x()
                        nc.tensor.matmul(
                            bcT[:],
                            ones_sb[HD : HD + 1, :],
                            rs[HD : HD + 1, n * 512 : (n + 1) * 512],
                            start=True,
                            stop=True,
                            tile_position=(96, 0),
                        )
                        nc.vector.tensor_tensor(
                            out=attn[
                                :,
                                h,
                                qc * 1024 + n * 512 : qc * 1024 + (n + 1) * 512,
                            ],
                            in0=o_sb[0:HD, n * 512 : (n + 1) * 512],
                            in1=bcT[0:HD, :],
                            op=AluOpType.mult,
                        )

                return norm_rest

            def attn_block(qc, h, prev_norm, blk):
                drain_deadline(blk)
                po = psum_po()
                if h == 0:
                    KTh = K3[0:HD, 0, :]
                    QTh = Q3[0:HD, 0, :]
                else:
                    KTh = KT[:, h, :]
                    QTh = QT[:, h, :]
                p_prev = None
                for kv in range(KV_T):
                    s = psum_s()
                    for n in range(2):
                        nc.tensor.matmul(
                            s[:, n * 512 : (n + 1) * 512],
                            KTh[:, kv * 128 : (kv + 1) * 128],
                            QTh[
                                :,
                                qc * 1024 + n * 512 : qc * 1024 + (n + 1) * 512,
                            ],
                            start=True,
                            stop=True,
                        )
                    p = sb.tile([128, 1024], BF16, tag="p", bufs=3, name="p")
                    nc.scalar.activation(
                        p[:], s[:], mybir.ActivationFunctionType.Exp,
                        scale=INV_SQRT_E,
                    )
                    if kv == 7 and prev_norm is not None:
                        prev_norm()
                    drain_one()
                    drain_one()
                    if p_prev is not None:
                        for n in range(2):
                            nc.tensor.matmul(
                                po[0:VW, n * 512 : (n + 1) * 512],
                                V[:, kv - 1, h * VW : (h + 1) * VW],
                                p_prev[:, n * 512 : (n + 1) * 512],
                                start=(kv == 1),
                                stop=False,
                            )
                    p_prev = p
                for n in range(2):
                    nc.tensor.matmul(
                        po[0:VW, n * 512 : (n + 1) * 512],
                        V[:, KV_T - 1, h * VW : (h + 1) * VW],
                        p_prev[:, n * 512 : (n + 1) * 512],
                        start=False,
                        stop=True,
                    )
                o_sb = sb.tile([HD + 1, 1024], F32, tag="osb", bufs=2, name="o_sb")
                nc.vector.tensor_copy(o_sb[:], po[0 : HD + 1, :])
                rs = sb.tile([HD + 1, 1024], BF16, tag="rs", bufs=2, name="rs")
                rf = sb.tile([HD + 1, 1024], F32, tag="rf", bufs=2, name="rf")
                rscr = sb.tile([HD + 1, 1024], F32, tag="rscr", bufs=2, name="rscr")
                nc.vector.reciprocal_approx_accurate(
                    out=rf[:], in_=o_sb[:], scratch=rscr[:]
                )
                nc.vector.tensor_copy(rs[HD : HD + 1, :], rf[HD : HD + 1, :])
                return norm_rest_for(o_sb, rs, h, qc)

            # ---------------- preamble ----------------
            for _ in range(55):
                wps = psum_x()
                nc.tensor.matmul(
                    wps[:], warm[:, 0:128], warm[:], start=True, stop=True
                )
            kp_unit(wk_sb, bk_sb, K3, xkv_sb, 0, 0)
            kp_unit(wq_sb, bq_sb, Q3, xq_sb, 0, 0)
            kp_unit(wq_sb, bq_sb, Q3, xq_sb, 0, 1)

            # ---------------- filler queues ----------------
            # order matters: block-0-critical units lead (V(t) before PV
            # uses it, K(h0) chunks before their kv range); repack units
            # are DMA-only and ride along with their producer chunks.
            def K_unit(t3, c):
                return lambda: kp_unit(wk_sb, bk_sb, K3, xkv_sb, t3, c)

            def Q_unit(t3, c):
                return lambda: kp_unit(wq_sb, bq_sb, Q3, xq_sb, t3, c)

            def RK(h, c):
                return lambda: repack_unit(K3, KT, nc.sync, h, c)

            def RQ(h, c):
                return lambda: repack_unit(Q3, QT, nc.scalar, h, c)

            def VU(t):
                return lambda: v_unit(t)

            blk0 = [
                K_unit(0, 1), VU(0), VU(1),
                K_unit(0, 2), VU(2), VU(3),
                K_unit(0, 3), VU(4), VU(5),
                VU(6), VU(7), VU(8), VU(9), VU(10), VU(11),
                VU(12), VU(13), VU(14), VU(15),
            ]
            blk1 = []
            for c in range(4):
                blk1 += [K_unit(1, c), RK(1, c)]
            for c in range(2):
                blk1 += [Q_unit(1, c), RQ(1, c)]
            blk2 = []
            for c in range(4):
                blk2 += [K_unit(2, c), RK(2, c)]
            for c in range(2):
                blk2 += [Q_unit(2, c), RQ(2, c)]
            blk3 = []
            for c in range(4):
                blk3 += [RK(3, c)]
            for c in range(2):
                blk3 += [RQ(3, c)]
            blk4 = []
            for c in range(2, 4):
                blk4 += [Q_unit(0, c), Q_unit(1, c), Q_unit(2, c),
                         RQ(1, c), RQ(2, c), RQ(3, c)]
            fillerA = deque(
                [(1, f) for f in blk0]
                + [(1, f) for f in blk1]
                + [(2, f) for f in blk2]
                + [(3, f) for f in blk3]
                + [(4, f) for f in blk4]
            )
            for qt in range(8):  # outproj for qc0
                fillerB.append(lambda qt=qt: op_unit_x(qt))
            for qt in range(8, QT_T):  # qc1 h0+h1 partials
                fillerC.append(lambda qt=qt: op_h01_unit(qt))

            # ---------------- main loop ----------------
            prev_norm = None
            for qc in range(2):
                for h in range(H_LOCAL):
                    blk = qc * 4 + h
                    if blk == 5:
                        b_unlocked[0] = True
                    if blk == 7:
                        c_unlocked[0] = True
                    prev_norm = attn_block(qc, h, prev_norm, blk)
            # tail: last norm, remaining fillers, outproj qc1
            prev_norm()
            while fillerA:
                fillerA.popleft()[1]()
            b_unlocked[0] = True
            while fillerB:
                fillerB.popleft()()
            c_unlocked[0] = True
            while fillerC:
                fillerC.popleft()()
            for i, qt in enumerate(range(8, QT_T)):
                m = i % 3
                if m == 0:
                    fa = psum_x()
                    fb = psum_x()
                    op_h23_unit(qt, fa[:], fb[:, 0:256])
                elif m == 1:
                    t = psum_s()
                    op_h23_unit(qt, t[:, 0:512], t[:, 512:768])
                else:
                    t = psum_po()
                    op_h23_unit(qt, t[:, 0:512], t[:, 512:768])

    nc.compile()
    return nc


_NC_CACHE = None


def _prep_inputs(x_query, x_kv, Wq, bq, Wk, bk, Wv, bv, Wo, bo):
    import ml_dtypes

    bf16 = ml_dtypes.bfloat16

    def pad_w(W_sl):
        # [384, 768] -> [768, 4, 97] with a zero ones-column, -> [768, 388]
        wp = np.zeros((E, H_LOCAL, VW), dtype=np.float32)
        wt = W_sl.T  # [768, 384]
        for h in range(H_LOCAL):
            wp[:, h, 0:HD] = wt[:, h * HD : (h + 1) * HD]
        return np.ascontiguousarray(wp.reshape(E, DV)).astype(bf16)

    in_maps = []
    for c in range(8):
        b, g = divmod(c, 2)
        sl = slice(g * D, (g + 1) * D)
        bve = np.zeros((DV,), dtype=np.float32)
        for h in range(H_LOCAL):
            bve[h * VW : h * VW + HD] = bv[sl][h * HD : (h + 1) * HD]
            bve[h * VW + HD] = 1.0
        consts = np.zeros((128, 6 + DV), dtype=np.float32)
        consts[:, 0:3] = bk[sl].reshape(3, 128).T
        consts[:, 3:6] = bq[sl].reshape(3, 128).T
        consts[:, 6 : 6 + DV] = bve[None, :]
        in_maps.append(
            {
                "xq_t": np.ascontiguousarray(x_query[b].T).astype(bf16),
                "xkv_t": np.ascontiguousarray(x_kv[b].T).astype(bf16),
                "wq_t": np.ascontiguousarray(Wq[sl, :].T).astype(bf16),
                "wk_t": np.ascontiguousarray(Wk[sl, :].T).astype(bf16),
                "wv_t": pad_w(Wv[sl, :]),
                "wo_t": np.ascontiguousarray(Wo[:, sl].T).astype(bf16),
                "consts_t": consts,
            }
        )
    return in_maps


def kernel(x_query, x_kv, Wq, bq, Wk, bk, Wv, bv, Wo, bo):
    global _NC_CACHE
    x_query = np.asarray(x_query, dtype=np.float32)
    x_kv = np.asarray(x_kv, dtype=np.float32)
    Wq = np.asarray(Wq, dtype=np.float32)
    Wk = np.asarray(Wk, dtype=np.float32)
    Wv = np.asarray(Wv, dtype=np.float32)
    Wo = np.asarray(Wo, dtype=np.float32)
    bq = np.asarray(bq, dtype=np.float32)
    bk = np.asarray(bk, dtype=np.float32)
    bv = np.asarray(bv, dtype=np.float32)
    bo = np.asarray(bo, dtype=np.float32)

    if _NC_CACHE is None:
        _NC_CACHE = build_nc()
    nc = _NC_CACHE

    in_maps = _prep_inputs(x_query, x_kv, Wq, bq, Wk, bk, Wv, bv, Wo, bo)

    trace = bool(int(os.environ.get("KERNEL_TRACE", "0")))
    res = bass_utils.run_bass_kernel_spmd(
        nc, in_maps, core_ids=list(range(8)), trace=trace
    )
    if trace:
        kernel.last_exec_time_ns = res.exec_time_ns
        kernel.last_results = res

    out = np.empty((B, NQ, E), dtype=np.float32)
    for b in range(B):
        out[b] = res.results[2 * b]["out"] + res.results[2 * b + 1]["out"] + bo
    return out


# revision 35
# speedup vs baseline: 1.0079x; 1.0079x over previous
"""Trainium2 Bass kernel for nn_CrossAttention (B=4, N=2048, E=768, H=8).

Sharding: 8 cores = 4 batches x 2 head-groups (4 heads of 96 dims each).
Each core computes its batch's attention for its 4 heads plus the partial
output projection; the host sums the two head-group partials per batch and
adds bo.

Design (v4, ~255us vs the 312us v0 baseline):
  - matmul stationaries are 128 columns where free (K/Q chunks, p tiles)
    for the compiler-automatic Fast Weight Load path; V uses exact 97-wide
    head blocks (96 dims + the softmax-rowsum ones column injected via the
    bias add) -- padding V cost more projection columns than FWL saved.
  - K/Q projections run 128-row packed (3 tiles of 128 dims, not 4 heads
    of 96) and SBUF->SBUF DMAs on the idle software-DGE ring repack heads
    1-3 to partition base 0; head 0 aliases the packed tile directly.
  - the scalar engine runs ONLY exp; the O^T copy and normalization run on
    DVE (reciprocal_approx_accurate over all 97 partitions -- custom DVE
    ops break at nonzero partition base), and the rowsum reciprocal row is
    broadcast with a 1-row bf16 matmul contracting on partition 96 via
    tile_position=(96,0).  No partition-shift DMA in the norm chain.
  - single software-pipelined emission: projection chunks, repacks and the
    qc0 out-projection are "filler units" drained two per kv-iteration
    into the attention loop so the PE never idles (pstate protection);
    qc1's out-projection is split h0+h1 early / h2+h3 tail, with the
    h0+h1 partials aliased onto xkv's dead SBUF storage.
  - PSUM is exactly 8 banks: tag s 2x[128,1024]f32, po 1x[128,1024],
    x 2x[128,512]; the tail ping-pongs over the then-dead s/po tags.
  - 55 dummy matmuls on a memset scratch tile keep the PE pstate warm
    through the ~20us initial DMA wait (the ramp resets on idle gaps).
  - DMA: ~115GB/s per ring, ~1us fixed cost per DMA, and Act-engine
    dispatches block the engine while its ring is busy -- so the scalar
    ring gets only the two early xq waves, gpsimd (software DGE) carries
    the weights + repacks, sync the rest; qc0 output stores rotate over
    all three rings, tail stores use the two hardware rings only.
"""

import os
import sys
import types
from collections import deque

import numpy as np

# ---------------------------------------------------------------------------
# NTFF profile hook (the agent image's antenv lacks axon_hooks; degrade OK)
# ---------------------------------------------------------------------------
def _install_ntff_hook():
    if "antenv.axon_hooks" in sys.modules:
        return
    try:
        hooks = types.ModuleType("antenv.axon_hooks")
        hooks._hook = None
        hooks.set_axon_ntff_profile_hook = lambda h: setattr(hooks, "_hook", h)
        hooks.get_axon_ntff_profile_hook = lambda: hooks._hook
        sys.modules["antenv.axon_hooks"] = hooks
        import antenv

        antenv.axon_hooks = hooks
        from trn_agent_boot.trn_boot import _ntff_profile_via_ctypes

        so = "/opt/axon/libaxon_pjrt.so"
        if os.path.exists(so):
            hooks.set_axon_ntff_profile_hook(_ntff_profile_via_ctypes(so))
    except Exception:
        pass


_install_ntff_hook()

import concourse.bacc as bacc
import concourse.tile as tile
import concourse.mybir as mybir
from concourse import bass_utils
from concourse.alu_op_type import AluOpType

F32 = mybir.dt.float32
F32R = mybir.dt.float32r
BF16 = mybir.dt.bfloat16

B = 4
NQ = 2048
NKV = 2048
E = 768
H_LOCAL = 4  # heads per core
HD = 96  # head dim
HP = 128  # padded head dim (FWL wants 128-wide stationaries)
D = H_LOCAL * HD  # 384 local proj dim
DP = H_LOCAL * HP  # 512 padded local proj dim
VW = HD + 1  # 97: per-head V block (96 dims + rowsum ones column)
DV = H_LOCAL * VW  # 388
ET = E // 128  # 6 contraction tiles
KV_T = NKV // 128  # 16 kv tiles
QT_T = NQ // 128  # 16 q tiles
INV_SQRT_E = 1.0 / float(np.sqrt(np.float32(E)))


def build_nc():
    nc = bacc.Bacc("TRN2", target_bir_lowering=False, debug=False)

    xq_t = nc.dram_tensor("xq_t", [E, NQ], BF16, kind="ExternalInput")
    xkv_t = nc.dram_tensor("xkv_t", [E, NKV], BF16, kind="ExternalInput")
    wq_t = nc.dram_tensor("wq_t", [E, D], BF16, kind="ExternalInput")
    wk_t = nc.dram_tensor("wk_t", [E, D], BF16, kind="ExternalInput")
    wv_t = nc.dram_tensor("wv_t", [E, DV], BF16, kind="ExternalInput")
    wo_t = nc.dram_tensor("wo_t", [D, E], BF16, kind="ExternalInput")
    consts_t = nc.dram_tensor("consts_t", [128, 6 + DV], F32, kind="ExternalInput")
    out = nc.dram_tensor("out", [NQ, E], F32, kind="ExternalOutput")

    with tile.TileContext(nc) as tc:
        with (
            nc.allow_low_precision(reason="bf16 matmuls and f32r broadcast"),
            tc.tile_pool(name="persist", bufs=1) as persist,
            tc.tile_pool(name="psum", bufs=1, space="PSUM") as pp,
            tc.tile_pool(name="sb", bufs=1) as sb,
        ):
            # ---------------- persistent SBUF tensors ----------------
            KT = persist.tile([HD, H_LOCAL, NKV], BF16)  # K^T per head
            QT = persist.tile([HD, H_LOCAL, NQ], BF16)  # Q^T per head
            # V: [kv-token, kv-tile, head-block(96 v dims + ones + 31 zeros)]
            V = persist.tile([128, KV_T, DV], BF16)
            attn = persist.tile([HD, H_LOCAL, NQ], BF16)  # normalized attn^T
            wo_sb = persist.tile([HD, H_LOCAL, E], BF16)
            wq_sb = persist.tile([128, ET, D], BF16)
            wk_sb = persist.tile([128, ET, D], BF16)
            wv_sb = persist.tile([128, ET, DV], BF16)
            K3 = persist.tile([128, 3, NKV], BF16)  # packed K^T staging
            Q3 = persist.tile([128, 3, NQ], BF16)  # packed Q^T staging
            consts_sb = persist.tile([128, 6 + DV], F32)
            bk_sb = consts_sb[:, 0:3]
            bq_sb = consts_sb[:, 3:6]
            bv_sb = consts_sb[:, 6 : 6 + DV]
            ones_sb = persist.tile([HD + 1, HP], BF16)  # row 96 = [1]*96+[0]*32
            xkv_sb = persist.tile([128, ET, NKV], BF16)
            # qc1 outproj h0+h1 partials alias xkv's storage (dead by then):
            # [128, 6, 2048] bf16 == [128, 6144] f32/2 == [128, 8, 768] f32
            ob01 = xkv_sb[:].bitcast(F32).rearrange("p t n -> p (t n)").rearrange(
                "p (a b) -> p a b", a=8
            )
            xq_sb = persist.tile([128, ET, NQ], BF16)

            # ---------------- input DMAs ----------------
            # Act-engine DMA dispatches block the engine while its ring is
            # busy, so the scalar ring gets ONLY the two early xq waves (ACT
            # must be free for exp by ~15us). gpsimd(software DGE) carries
            # the weights, sync everything else.
            nc.gpsimd.dma_start(
                wq_sb[:], wq_t[:].rearrange("(t p) n -> p t n", p=128)
            )
            nc.gpsimd.dma_start(
                wk_sb[:], wk_t[:].rearrange("(t p) n -> p t n", p=128)
            )
            nc.gpsimd.dma_start(
                wv_sb[:], wv_t[:].rearrange("(t p) n -> p t n", p=128)
            )
            nc.scalar.dma_start(
                xq_sb[:, 0:3, 0:1024],
                xq_t[0:384, 0:1024].rearrange("(t p) n -> p t n", p=128),
            )
            nc.scalar.dma_start(
                xq_sb[:, 3:6, 0:1024],
                xq_t[384:768, 0:1024].rearrange("(t p) n -> p t n", p=128),
            )
            nc.sync.dma_start(
                xkv_sb[:, 0:3, 0:1024],
                xkv_t[0:384, 0:1024].rearrange("(t p) n -> p t n", p=128),
            )
            nc.sync.dma_start(
                xkv_sb[:, 3:6, 0:1024],
                xkv_t[384:768, 0:1024].rearrange("(t p) n -> p t n", p=128),
            )
            nc.sync.dma_start(consts_sb[:], consts_t[:])
            nc.sync.dma_start(
                xkv_sb[:, :, 1024:2048],
                xkv_t[:, 1024:2048].rearrange("(t p) n -> p t n", p=128),
            )
            nc.sync.dma_start(
                xq_sb[:, :, 1024:2048],
                xq_t[:, 1024:2048].rearrange("(t p) n -> p t n", p=128),
            )
            nc.sync.dma_start(
                wo_sb[:], wo_t[:].rearrange("(h p) n -> p h n", p=HD)
            )
            nc.vector.memset(ones_sb[HD : HD + 1, 0:HD], 1.0)
            nc.vector.memset(ones_sb[HD : HD + 1, HD:HP], 0.0)

            # ---------------- PE warm-up ----------------
            # The PE pstate ramp (full speed only after ~3us of continuous
            # busy) resets on every idle gap; the first real matmul waits
            # ~20us for DMA. Keep the array warm through the wait with
            # dummy matmuls on a memset scratch tile (no DMA deps), sized
            # to end roughly when the first input wave lands.
            warm = persist.tile([128, 512], BF16)
            nc.vector.memset(warm[:], 0.0)

            # ---------------- psum slot helpers ----------------
            # tags: s 2x[128,1024], po 1x[128,1024], x 2x[128,512]: 8 banks
            def psum_s():
                return pp.tile([128, 1024], F32, tag="s", bufs=2, name="ps_s")

            def psum_po():
                return pp.tile([128, 1024], F32, tag="po", bufs=1, name="ps_po")

            def psum_x():
                return pp.tile([128, 512], F32, tag="x", bufs=2, name="ps_x")

            # ---------------- projection / outproj unit emitters ----------
            def kp_unit(w_sb, b3, dst3, x_sb, t3, c):
                # dst3[:, t3, c*512:(c+1)*512] = w[t3-block] @ x^T + b
                ps = psum_x()
                for e in range(ET):
                    nc.tensor.matmul(
                        ps[:],
                        w_sb[:, e, t3 * 128 : (t3 + 1) * 128],
                        x_sb[:, e, c * 512 : (c + 1) * 512],
                        start=(e == 0),
                        stop=(e == ET - 1),
                    )
                nc.vector.tensor_scalar_add(
                    out=dst3[:, t3, c * 512 : (c + 1) * 512],
                    in0=ps[:],
                    scalar1=b3[:, t3 : t3 + 1],
                )

            def repack_unit(src3, dst, ring, h, c):
                ring = nc.gpsimd
                # head h dims (96h..96h+95) -> dst[:, h, c-chunk], base 0
                cs = slice(c * 512, (c + 1) * 512)
                lo = h * HD
                t_lo, r_lo = divmod(lo, 128)
                n0 = min(128 - r_lo, HD)
                ring.dma_start(
                    dst[0:n0, h, cs], src3[r_lo : r_lo + n0, t_lo, cs]
                )
                if n0 < HD:
                    ring.dma_start(
                        dst[n0:HD, h, cs],
                        src3[0 : HD - n0, t_lo + 1, cs],
                    )

            def v_unit(t):
                # V[:, t, :] = x_t @ Wv^T + bve  (bve carries the ones column)
                ps = psum_x()
                for e in range(ET):
                    nc.tensor.matmul(
                        ps[:, 0:DV],
                        xkv_sb[:, e, t * 128 : (t + 1) * 128],
                        wv_sb[:, e, :],
                        start=(e == 0),
                        stop=(e == ET - 1),
                    )
                nc.vector.tensor_tensor(
                    out=V[:, t, :],
                    in0=ps[:, 0:DV],
                    in1=bv_sb[:],
                    op=AluOpType.add,
                )

            ob_i = [0]

            def op_unit(qt, fa, fb):
                # out[qt] = attn^T_qt.T @ Wo^T  (accumulate 4 heads)
                for h in range(H_LOCAL):
                    nc.tensor.matmul(
                        fa,
                        attn[:, h, qt * 128 : (qt + 1) * 128],
                        wo_sb[:, h, 0:512],
                        start=(h == 0),
                        stop=(h == H_LOCAL - 1),
                    )
                for h in range(H_LOCAL):
                    nc.tensor.matmul(
                        fb,
                        attn[:, h, qt * 128 : (qt + 1) * 128],
                        wo_sb[:, h, 512:768],
                        start=(h == 0),
                        stop=(h == H_LOCAL - 1),
                    )
                ob = sb.tile([128, E], F32, tag="ob", bufs=2, name="ob")
                nc.vector.tensor_copy(ob[:, 0:512], fa)
                nc.vector.tensor_copy(ob[:, 512:768], fb)
                ring = (nc.sync, nc.scalar, nc.gpsimd)[ob_i[0] % 3]
                ob_i[0] += 1
                ring.dma_start(out[qt * 128 : (qt + 1) * 128, :], ob[:])

            def op_unit_x(qt):
                fa = psum_x()
                fb = psum_x()
                op_unit(qt, fa[:], fb[:, 0:256])

            def op_h01_unit(qt):
                # heads 0-1 accumulation for qc1's out tiles
                fa = psum_x()
                fb = psum_x()
                for h in range(2):
                    nc.tensor.matmul(
                        fa[:],
                        attn[:, h, qt * 128 : (qt + 1) * 128],
                        wo_sb[:, h, 0:512],
                        start=(h == 0),
                        stop=(h == 1),
                    )
                    nc.tensor.matmul(
                        fb[:, 0:256],
                        attn[:, h, qt * 128 : (qt + 1) * 128],
                        wo_sb[:, h, 512:768],
                        start=(h == 0),
                        stop=(h == 1),
                    )
                nc.vector.tensor_copy(ob01[:, qt - 8, 0:512], fa[:])
                nc.vector.tensor_copy(ob01[:, qt - 8, 512:768], fb[:, 0:256])

            def op_h23_unit(qt, fa, fb):
                for h in range(2, 4):
                    nc.tensor.matmul(
                        fa,
                        attn[:, h, qt * 128 : (qt + 1) * 128],
                        wo_sb[:, h, 0:512],
                        start=(h == 2),
                        stop=(h == 3),
                    )
                    nc.tensor.matmul(
                        fb,
                        attn[:, h, qt * 128 : (qt + 1) * 128],
                        wo_sb[:, h, 512:768],
                        start=(h == 2),
                        stop=(h == 3),
                    )
                ob = sb.tile([128, E], F32, tag="ob", bufs=2, name="ob")
                nc.vector.tensor_tensor(
                    out=ob[:, 0:512], in0=fa, in1=ob01[:, qt - 8, 0:512],
                    op=AluOpType.add,
                )
                nc.vector.tensor_tensor(
                    out=ob[:, 512:768], in0=fb, in1=ob01[:, qt - 8, 512:768],
                    op=AluOpType.add,
                )
                ring = (nc.sync, nc.scalar, nc.gpsimd)[ob_i[0] % 3]
                ob_i[0] += 1
                ring.dma_start(out[qt * 128 : (qt + 1) * 128, :], ob[:])

            # ---------------- filler machinery ----------------
            fillerA = deque()  # projection units: (deadline_block, fn)
            fillerB = deque()  # outproj units for qc0 (unlocked at block 5)
            fillerC = deque()  # qc1 outproj h0+h1 partials (unlocked blk 7)
            b_unlocked = [False]
            c_unlocked = [False]

            def drain_one():
                if fillerA:
                    fillerA.popleft()[1]()
                elif b_unlocked[0] and fillerB:
                    fillerB.popleft()()
                elif c_unlocked[0] and fillerC:
                    fillerC.popleft()()

            def drain_deadline(blk):
                while fillerA and fillerA[0][0] <= blk:
                    fillerA.popleft()[1]()

            # ---------------- attention block ----------------
            def norm_rest_for(o_sb, rs, h, qc):
                def norm_rest():
                    for n in range(2):
                        bcT = psum_x()
                        nc.tensor.matmul(
                            bcT[:],
                            ones_sb[HD : HD + 1, :],
                            rs[HD : HD + 1, n * 512 : (n + 1) * 512],
                            start=True,
                            stop=True,
                            tile_position=(96, 0),
                        )
                        nc.vector.tensor_tensor(
                            out=attn[
                                :,
                                h,
                                qc * 1024 + n * 512 : qc * 1024 + (n + 1) * 512,
                            ],
                            in0=o_sb[0:HD, n * 512 : (n + 1) * 512],
                            in1=bcT[0:HD, :],
                            op=AluOpType.mult,
                        )

                return norm_rest

            def attn_block(qc, h, prev_norm, blk):
                drain_deadline(blk)
                po = psum_po()
                if h == 0:
                    KTh = K3[0:HD, 0, :]
                    QTh = Q3[0:HD, 0, :]
                else:
                    KTh = KT[:, h, :]
                    QTh = QT[:, h, :]
                p_prev = None
                for kv in range(KV_T):
                    s = psum_s()
                    for n in range(2):
                        nc.tensor.matmul(
                            s[:, n * 512 : (n + 1) * 512],
                            KTh[:, kv * 128 : (kv + 1) * 128],
                            QTh[
                                :,
                                qc * 1024 + n * 512 : qc * 1024 + (n + 1) * 512,
                            ],
                            start=True,
                            stop=True,
                        )
                    p = sb.tile([128, 1024], BF16, tag="p", bufs=3, name="p")
                    nc.scalar.activation(
                        p[:], s[:], mybir.ActivationFunctionType.Exp,
                        scale=INV_SQRT_E,
                    )
                    if kv == 7 and prev_norm is not None:
                        prev_norm()
                    drain_one()
                    drain_one()
                    if p_prev is not None:
                        for n in range(2):
                            nc.tensor.matmul(
                                po[0:VW, n * 512 : (n + 1) * 512],
                                V[:, kv - 1, h * VW : (h + 1) * VW],
                                p_prev[:, n * 512 : (n + 1) * 512],
                                start=(kv == 1),
                                stop=False,
                            )
                    p_prev = p
                for n in range(2):
                    nc.tensor.matmul(
                        po[0:VW, n * 512 : (n + 1) * 512],
                        V[:, KV_T - 1, h * VW : (h + 1) * VW],
                        p_prev[:, n * 512 : (n + 1) * 512],
                        start=False,
                        stop=True,
                    )
                o_sb = sb.tile([HD + 1, 1024], F32, tag="osb", bufs=2, name="o_sb")
                nc.vector.tensor_copy(o_sb[:], po[0 : HD + 1, :])
                rs = sb.tile([HD + 1, 1024], BF16, tag="rs", bufs=2, name="rs")
                rf = sb.tile([HD + 1, 1024], F32, tag="rf", bufs=2, name="rf")
                rscr = sb.tile([HD + 1, 1024], F32, tag="rscr", bufs=2, name="rscr")
                nc.vector.reciprocal_approx_accurate(
                    out=rf[:], in_=o_sb[:], scratch=rscr[:]
                )
                nc.vector.tensor_copy(rs[HD : HD + 1, :], rf[HD : HD + 1, :])
                return norm_rest_for(o_sb, rs, h, qc)

            # ---------------- preamble ----------------
            for _ in range(55):
                wps = psum_x()
                nc.tensor.matmul(
                    wps[:], warm[:, 0:128], warm[:], start=True, stop=True
                )
            kp_unit(wk_sb, bk_sb, K3, xkv_sb, 0, 0)
            kp_unit(wq_sb, bq_sb, Q3, xq_sb, 0, 0)
            kp_unit(wq_sb, bq_sb, Q3, xq_sb, 0, 1)

            # ---------------- filler queues ----------------
            # order matters: block-0-critical units lead (V(t) before PV
            # uses it, K(h0) chunks before their kv range); repack units
            # are DMA-only and ride along with their producer chunks.
            def K_unit(t3, c):
                return lambda: kp_unit(wk_sb, bk_sb, K3, xkv_sb, t3, c)

            def Q_unit(t3, c):
                return lambda: kp_unit(wq_sb, bq_sb, Q3, xq_sb, t3, c)

            def RK(h, c):
                return lambda: repack_unit(K3, KT, nc.sync, h, c)

            def RQ(h, c):
                return lambda: repack_unit(Q3, QT, nc.scalar, h, c)

            def VU(t):
                return lambda: v_unit(t)

            blk0 = [
                K_unit(0, 1), VU(0), VU(1),
                K_unit(0, 2), VU(2), VU(3),
                K_unit(0, 3), VU(4), VU(5),
                VU(6), VU(7), VU(8), VU(9), VU(10), VU(11),
                VU(12), VU(13), VU(14), VU(15),
            ]
            blk1 = []
            for c in range(4):
                blk1 += [K_unit(1, c), RK(1, c)]
            for c in range(2):
                blk1 += [Q_unit(1, c), RQ(1, c)]
            blk2 = []
            for c in range(4):
                blk2 += [K_unit(2, c), RK(2, c)]
            for c in range(2):
                blk2 += [Q_unit(2, c), RQ(2, c)]
            blk3 = []
            for c in range(4):
                blk3 += [RK(3, c)]
            for c in range(2):
                blk3 += [RQ(3, c)]
            blk4 = []
            for c in range(2, 4):
                blk4 += [Q_unit(0, c), Q_unit(1, c), Q_unit(2, c),
                         RQ(1, c), RQ(2, c), RQ(3, c)]
            fillerA = deque(
                [(1, f) for f in blk0]
                + [(1, f) for f in blk1]
                + [(2, f) for f in blk2]
                + [(3, f) for f in blk3]
                + [(4, f) for f in blk4]
            )
            for qt in range(8):  # outproj for qc0
                fillerB.append(lambda qt=qt: op_unit_x(qt))
            for qt in range(8, QT_T):  # qc1 h0+h1 partials
                fillerC.append(lambda qt=qt: op_h01_unit(qt))

            # ---------------- main loop ----------------
            prev_norm = None
            for qc in range(2):
                for h in range(H_LOCAL):
                    blk = qc * 4 + h
                    if blk == 5:
                        b_unlocked[0] = True
                    if blk == 7:
                        c_unlocked[0] = True
                    prev_norm = attn_block(qc, h, prev_norm, blk)
            # tail: last norm, remaining fillers, outproj qc1
            prev_norm()
            while fillerA:
                fillerA.popleft()[1]()
            b_unlocked[0] = True
            while fillerB:
                fillerB.popleft()()
            c_unlocked[0] = True
            while fillerC:
                fillerC.popleft()()
            for i, qt in enumerate(range(8, QT_T)):
                m = i % 3
                if m == 0:
                    fa = psum_x()
                    fb = psum_x()
                    op_h23_unit(qt, fa[:], fb[:, 0:256])
                elif m == 1:
                    t = psum_s()
                    op_h23_unit(qt, t[:, 0:512], t[:, 512:768])
                else:
                    t = psum_po()
                    op_h23_unit(qt, t[:, 0:512], t[:, 512:768])

    nc.compile()
    return nc


_NC_CACHE = None


def _prep_inputs(x_query, x_kv, Wq, bq, Wk, bk, Wv, bv, Wo, bo):
    import ml_dtypes

    bf16 = ml_dtypes.bfloat16

    def pad_w(W_sl):
        # [384, 768] -> [768, 4, 97] with a zero ones-column, -> [768, 388]
        wp = np.zeros((E, H_LOCAL, VW), dtype=np.float32)
        wt = W_sl.T  # [768, 384]
        for h in range(H_LOCAL):
            wp[:, h, 0:HD] = wt[:, h * HD : (h + 1) * HD]
        return np.ascontiguousarray(wp.reshape(E, DV)).astype(bf16)

    in_maps = []
    for c in range(8):
        b, g = divmod(c, 2)
        sl = slice(g * D, (g + 1) * D)
        bve = np.zeros((DV,), dtype=np.float32)
        for h in range(H_LOCAL):
            bve[h * VW : h * VW + HD] = bv[sl][h * HD : (h + 1) * HD]
            bve[h * VW + HD] = 1.0
        consts = np.zeros((128, 6 + DV), dtype=np.float32)
        consts[:, 0:3] = bk[sl].reshape(3, 128).T
        consts[:, 3:6] = bq[sl].reshape(3, 128).T
        consts[:, 6 : 6 + DV] = bve[None, :]
        in_maps.append(
            {
                "xq_t": np.ascontiguousarray(x_query[b].T).astype(bf16),
                "xkv_t": np.ascontiguousarray(x_kv[b].T).astype(bf16),
                "wq_t": np.ascontiguousarray(Wq[sl, :].T).astype(bf16),
                "wk_t": np.ascontiguousarray(Wk[sl, :].T).astype(bf16),
                "wv_t": pad_w(Wv[sl, :]),
                "wo_t": np.ascontiguousarray(Wo[:, sl].T).astype(bf16),
                "consts_t": consts,
            }
        )
    return in_maps


def kernel(x_query, x_kv, Wq, bq, Wk, bk, Wv, bv, Wo, bo):
    global _NC_CACHE
    x_query = np.asarray(x_query, dtype=np.float32)
    x_kv = np.asarray(x_kv, dtype=np.float32)
    Wq = np.asarray(Wq, dtype=np.float32)
    Wk = np.asarray(Wk, dtype=np.float32)
    Wv = np.asarray(Wv, dtype=np.float32)
    Wo = np.asarray(Wo, dtype=np.float32)
    bq = np.asarray(bq, dtype=np.float32)
    bk = np.asarray(bk, dtype=np.float32)
    bv = np.asarray(bv, dtype=np.float32)
    bo = np.asarray(bo, dtype=np.float32)

    if _NC_CACHE is None:
        _NC_CACHE = build_nc()
    nc = _NC_CACHE

    in_maps = _prep_inputs(x_query, x_kv, Wq, bq, Wk, bk, Wv, bv, Wo, bo)

    trace = bool(int(os.environ.get("KERNEL_TRACE", "0")))
    res = bass_utils.run_bass_kernel_spmd(
        nc, in_maps, core_ids=list(range(8)), trace=trace
    )
    if trace:
        kernel.last_exec_time_ns = res.exec_time_ns
        kernel.last_results = res

    out = np.empty((B, NQ, E), dtype=np.float32)
    for b in range(B):
        out[b] = res.results[2 * b]["out"] + res.results[2 * b + 1]["out"] + bo
    return out


# revision 36
# speedup vs baseline: 1.0194x; 1.0114x over previous
"""Trainium2 Bass kernel for nn_CrossAttention (B=4, N=2048, E=768, H=8).

Sharding: 8 cores = 4 batches x 2 head-groups (4 heads of 96 dims each).
Each core computes its batch's attention for its 4 heads plus the partial
output projection; the host sums the two head-group partials per batch and
adds bo.

Design (v4, ~255us vs the 312us v0 baseline):
  - matmul stationaries are 128 columns where free (K/Q chunks, p tiles)
    for the compiler-automatic Fast Weight Load path; V uses exact 97-wide
    head blocks (96 dims + the softmax-rowsum ones column injected via the
    bias add) -- padding V cost more projection columns than FWL saved.
  - K/Q projections run 128-row packed (3 tiles of 128 dims, not 4 heads
    of 96) and SBUF->SBUF DMAs on the idle software-DGE ring repack heads
    1-3 to partition base 0; head 0 aliases the packed tile directly.
  - the scalar engine runs ONLY exp; the O^T copy and normalization run on
    DVE (reciprocal_approx_accurate over all 97 partitions -- custom DVE
    ops break at nonzero partition base), and the rowsum reciprocal row is
    broadcast with a 1-row bf16 matmul contracting on partition 96 via
    tile_position=(96,0).  No partition-shift DMA in the norm chain.
  - single software-pipelined emission: projection chunks, repacks and the
    qc0 out-projection are "filler units" drained two per kv-iteration
    into the attention loop so the PE never idles (pstate protection);
    qc1's out-projection is split h0+h1 early / h2+h3 tail, with the
    h0+h1 partials aliased onto xkv's dead SBUF storage.
  - PSUM is exactly 8 banks: tag s 2x[128,1024]f32, po 1x[128,1024],
    x 2x[128,512]; the tail ping-pongs over the then-dead s/po tags.
  - 55 dummy matmuls on a memset scratch tile keep the PE pstate warm
    through the ~20us initial DMA wait (the ramp resets on idle gaps).
  - DMA: ~115GB/s per ring, ~1us fixed cost per DMA, and Act-engine
    dispatches block the engine while its ring is busy -- so the scalar
    ring gets only the two early xq waves, gpsimd (software DGE) carries
    the weights + repacks, sync the rest; qc0 output stores rotate over
    all three rings, tail stores use the two hardware rings only.
"""

import os
import sys
import types
from collections import deque

import numpy as np

# ---------------------------------------------------------------------------
# NTFF profile hook (the agent image's antenv lacks axon_hooks; degrade OK)
# ---------------------------------------------------------------------------
def _install_ntff_hook():
    if "antenv.axon_hooks" in sys.modules:
        return
    try:
        hooks = types.ModuleType("antenv.axon_hooks")
        hooks._hook = None
        hooks.set_axon_ntff_profile_hook = lambda h: setattr(hooks, "_hook", h)
        hooks.get_axon_ntff_profile_hook = lambda: hooks._hook
        sys.modules["antenv.axon_hooks"] = hooks
        import antenv

        antenv.axon_hooks = hooks
        from trn_agent_boot.trn_boot import _ntff_profile_via_ctypes

        so = "/opt/axon/libaxon_pjrt.so"
        if os.path.exists(so):
            hooks.set_axon_ntff_profile_hook(_ntff_profile_via_ctypes(so))
    except Exception:
        pass


_install_ntff_hook()

import concourse.bacc as bacc
import concourse.tile as tile
import concourse.mybir as mybir
from concourse import bass_utils
from concourse.alu_op_type import AluOpType

F32 = mybir.dt.float32
F32R = mybir.dt.float32r
BF16 = mybir.dt.bfloat16

B = 4
NQ = 2048
NKV = 2048
E = 768
H_LOCAL = 4  # heads per core
HD = 96  # head dim
HP = 128  # padded head dim (FWL wants 128-wide stationaries)
D = H_LOCAL * HD  # 384 local proj dim
DP = H_LOCAL * HP  # 512 padded local proj dim
VW = HD + 1  # 97: per-head V block (96 dims + rowsum ones column)
DV = H_LOCAL * VW  # 388
ET = E // 128  # 6 contraction tiles
KV_T = NKV // 128  # 16 kv tiles
QT_T = NQ // 128  # 16 q tiles
INV_SQRT_E = 1.0 / float(np.sqrt(np.float32(E)))


def build_nc():
    nc = bacc.Bacc("TRN2", target_bir_lowering=False, debug=False)

    xq_t = nc.dram_tensor("xq_t", [E, NQ], BF16, kind="ExternalInput")
    xkv_t = nc.dram_tensor("xkv_t", [E, NKV], BF16, kind="ExternalInput")
    wq_t = nc.dram_tensor("wq_t", [E, D], BF16, kind="ExternalInput")
    wk_t = nc.dram_tensor("wk_t", [E, D], BF16, kind="ExternalInput")
    wv_t = nc.dram_tensor("wv_t", [E, DV], BF16, kind="ExternalInput")
    wo_t = nc.dram_tensor("wo_t", [D, E], BF16, kind="ExternalInput")
    consts_t = nc.dram_tensor("consts_t", [128, 6 + DV], F32, kind="ExternalInput")
    out = nc.dram_tensor("out", [NQ, E], F32, kind="ExternalOutput")

    with tile.TileContext(nc) as tc:
        with (
            nc.allow_low_precision(reason="bf16 matmuls and f32r broadcast"),
            tc.tile_pool(name="persist", bufs=1) as persist,
            tc.tile_pool(name="psum", bufs=1, space="PSUM") as pp,
            tc.tile_pool(name="sb", bufs=1) as sb,
        ):
            # ---------------- persistent SBUF tensors ----------------
            KT = persist.tile([HD, H_LOCAL, NKV], BF16)  # K^T per head
            QT = persist.tile([HD, H_LOCAL, NQ], BF16)  # Q^T per head
            # V: [kv-token, kv-tile, head-block(96 v dims + ones + 31 zeros)]
            V = persist.tile([128, KV_T, DV], BF16)
            attn = persist.tile([HD, H_LOCAL, NQ], BF16)  # normalized attn^T
            wo_sb = persist.tile([HD, H_LOCAL, E], BF16)
            wq_sb = persist.tile([128, ET, D], BF16)
            wk_sb = persist.tile([128, ET, D], BF16)
            wv_sb = persist.tile([128, ET, DV], BF16)
            K3 = persist.tile([128, 3, NKV], BF16)  # packed K^T staging
            Q3 = persist.tile([128, 3, NQ], BF16)  # packed Q^T staging
            consts_sb = persist.tile([128, 6 + DV], F32)
            bk_sb = consts_sb[:, 0:3]
            bq_sb = consts_sb[:, 3:6]
            bv_sb = consts_sb[:, 6 : 6 + DV]
            ones_sb = persist.tile([HD + 1, HP], BF16)  # row 96 = [1]*96+[0]*32
            xkv_sb = persist.tile([128, ET, NKV], BF16)
            # qc1 outproj h0+h1 partials alias xkv's storage (dead by then):
            # [128, 6, 2048] bf16 == [128, 6144] f32/2 == [128, 8, 768] f32
            ob01 = xkv_sb[:].bitcast(F32).rearrange("p t n -> p (t n)").rearrange(
                "p (a b) -> p a b", a=8
            )
            xq_sb = persist.tile([128, ET, NQ], BF16)

            # ---------------- input DMAs ----------------
            # Act-engine DMA dispatches block the engine while its ring is
            # busy, so the scalar ring gets ONLY the two early xq waves (ACT
            # must be free for exp by ~15us). gpsimd(software DGE) carries
            # the weights, sync everything else.
            nc.gpsimd.dma_start(
                wq_sb[:], wq_t[:].rearrange("(t p) n -> p t n", p=128)
            )
            nc.gpsimd.dma_start(
                wk_sb[:], wk_t[:].rearrange("(t p) n -> p t n", p=128)
            )
            nc.gpsimd.dma_start(
                wv_sb[:], wv_t[:].rearrange("(t p) n -> p t n", p=128)
            )
            nc.scalar.dma_start(
                xq_sb[:, 0:3, 0:1024],
                xq_t[0:384, 0:1024].rearrange("(t p) n -> p t n", p=128),
            )
            nc.scalar.dma_start(
                xq_sb[:, 3:6, 0:1024],
                xq_t[384:768, 0:1024].rearrange("(t p) n -> p t n", p=128),
            )
            nc.sync.dma_start(
                xkv_sb[:, 0:3, 0:1024],
                xkv_t[0:384, 0:1024].rearrange("(t p) n -> p t n", p=128),
            )
            nc.sync.dma_start(
                xkv_sb[:, 3:6, 0:1024],
                xkv_t[384:768, 0:1024].rearrange("(t p) n -> p t n", p=128),
            )
            nc.sync.dma_start(consts_sb[:], consts_t[:])
            nc.sync.dma_start(
                xkv_sb[:, :, 1024:2048],
                xkv_t[:, 1024:2048].rearrange("(t p) n -> p t n", p=128),
            )
            nc.sync.dma_start(
                xq_sb[:, :, 1024:2048],
                xq_t[:, 1024:2048].rearrange("(t p) n -> p t n", p=128),
            )
            nc.sync.dma_start(
                wo_sb[:], wo_t[:].rearrange("(h p) n -> p h n", p=HD)
            )
            nc.vector.memset(ones_sb[HD : HD + 1, 0:HD], 1.0)
            nc.vector.memset(ones_sb[HD : HD + 1, HD:HP], 0.0)

            # ---------------- PE warm-up ----------------
            # The PE pstate ramp (full speed only after ~3us of continuous
            # busy) resets on every idle gap; the first real matmul waits
            # ~20us for DMA. Keep the array warm through the wait with
            # dummy matmuls on a memset scratch tile (no DMA deps), sized
            # to end roughly when the first input wave lands.
            warm = persist.tile([128, 512], BF16)
            nc.vector.memset(warm[:], 0.0)

            # ---------------- psum slot helpers ----------------
            # tags: s 2x[128,1024], po 1x[128,1024], x 2x[128,512]: 8 banks
            def psum_s():
                return pp.tile([128, 1024], F32, tag="s", bufs=2, name="ps_s")

            def psum_po():
                return pp.tile([128, 1024], F32, tag="po", bufs=1, name="ps_po")

            def psum_x():
                return pp.tile([128, 512], F32, tag="x", bufs=2, name="ps_x")

            # ---------------- projection / outproj unit emitters ----------
            def kp_unit(w_sb, b3, dst3, x_sb, t3, c):
                # dst3[:, t3, c*512:(c+1)*512] = w[t3-block] @ x^T + b
                ps = psum_x()
                for e in range(ET):
                    nc.tensor.matmul(
                        ps[:],
                        w_sb[:, e, t3 * 128 : (t3 + 1) * 128],
                        x_sb[:, e, c * 512 : (c + 1) * 512],
                        start=(e == 0),
                        stop=(e == ET - 1),
                    )
                nc.vector.tensor_scalar_add(
                    out=dst3[:, t3, c * 512 : (c + 1) * 512],
                    in0=ps[:],
                    scalar1=b3[:, t3 : t3 + 1],
                )

            def repack_unit(src3, dst, ring, h, c):
                ring = nc.gpsimd
                # head h dims (96h..96h+95) -> dst[:, h, c-chunk], base 0
                cs = slice(c * 512, (c + 1) * 512)
                lo = h * HD
                t_lo, r_lo = divmod(lo, 128)
                n0 = min(128 - r_lo, HD)
                ring.dma_start(
                    dst[0:n0, h, cs], src3[r_lo : r_lo + n0, t_lo, cs]
                )
                if n0 < HD:
                    ring.dma_start(
                        dst[n0:HD, h, cs],
                        src3[0 : HD - n0, t_lo + 1, cs],
                    )

            def v_unit(t):
                # V[:, t, :] = x_t @ Wv^T + bve  (bve carries the ones column)
                ps = psum_x()
                for e in range(ET):
                    nc.tensor.matmul(
                        ps[:, 0:DV],
                        xkv_sb[:, e, t * 128 : (t + 1) * 128],
                        wv_sb[:, e, :],
                        start=(e == 0),
                        stop=(e == ET - 1),
                    )
                nc.vector.tensor_tensor(
                    out=V[:, t, :],
                    in0=ps[:, 0:DV],
                    in1=bv_sb[:],
                    op=AluOpType.add,
                )

            ob_i = [0]

            def op_unit(qt, fa, fb):
                # out[qt] = attn^T_qt.T @ Wo^T  (accumulate 4 heads)
                for h in range(H_LOCAL):
                    nc.tensor.matmul(
                        fa,
                        attn[:, h, qt * 128 : (qt + 1) * 128],
                        wo_sb[:, h, 0:512],
                        start=(h == 0),
                        stop=(h == H_LOCAL - 1),
                    )
                for h in range(H_LOCAL):
                    nc.tensor.matmul(
                        fb,
                        attn[:, h, qt * 128 : (qt + 1) * 128],
                        wo_sb[:, h, 512:768],
                        start=(h == 0),
                        stop=(h == H_LOCAL - 1),
                    )
                ob = sb.tile([128, E], F32, tag="ob", bufs=2, name="ob")
                nc.vector.tensor_copy(ob[:, 0:512], fa)
                nc.vector.tensor_copy(ob[:, 512:768], fb)
                ring = (nc.sync, nc.scalar, nc.gpsimd)[ob_i[0] % 3]
                ob_i[0] += 1
                ring.dma_start(out[qt * 128 : (qt + 1) * 128, :], ob[:])

            def op_unit_x(qt):
                fa = psum_x()
                fb = psum_x()
                op_unit(qt, fa[:], fb[:, 0:256])

            def op_h01_unit(qt):
                # heads 0-1 accumulation for qc1's out tiles
                fa = psum_x()
                fb = psum_x()
                for h in range(2):
                    nc.tensor.matmul(
                        fa[:],
                        attn[:, h, qt * 128 : (qt + 1) * 128],
                        wo_sb[:, h, 0:512],
                        start=(h == 0),
                        stop=(h == 1),
                    )
                    nc.tensor.matmul(
                        fb[:, 0:256],
                        attn[:, h, qt * 128 : (qt + 1) * 128],
                        wo_sb[:, h, 512:768],
                        start=(h == 0),
                        stop=(h == 1),
                    )
                nc.vector.tensor_copy(ob01[:, qt - 8, 0:512], fa[:])
                nc.vector.tensor_copy(ob01[:, qt - 8, 512:768], fb[:, 0:256])

            def op_h23_unit(qt, fa, fb):
                for h in range(2, 4):
                    nc.tensor.matmul(
                        fa,
                        attn[:, h, qt * 128 : (qt + 1) * 128],
                        wo_sb[:, h, 0:512],
                        start=(h == 2),
                        stop=(h == 3),
                    )
                    nc.tensor.matmul(
                        fb,
                        attn[:, h, qt * 128 : (qt + 1) * 128],
                        wo_sb[:, h, 512:768],
                        start=(h == 2),
                        stop=(h == 3),
                    )
                ob = sb.tile([128, E], F32, tag="ob", bufs=2, name="ob")
                nc.vector.tensor_tensor(
                    out=ob[:, 0:512], in0=fa, in1=ob01[:, qt - 8, 0:512],
                    op=AluOpType.add,
                )
                nc.vector.tensor_tensor(
                    out=ob[:, 512:768], in0=fb, in1=ob01[:, qt - 8, 512:768],
                    op=AluOpType.add,
                )
                ring = (nc.sync, nc.scalar, nc.gpsimd)[ob_i[0] % 3]
                ob_i[0] += 1
                ring.dma_start(out[qt * 128 : (qt + 1) * 128, :], ob[:])

            # ---------------- filler machinery ----------------
            fillerA = deque()  # projection units: (deadline_block, fn)
            fillerB = deque()  # outproj units for qc0 (unlocked at block 5)
            fillerC = deque()  # qc1 outproj h0+h1 partials (unlocked blk 7)
            b_unlocked = [False]
            c_unlocked = [False]

            def drain_one():
                if fillerA:
                    fillerA.popleft()[1]()
                elif b_unlocked[0] and fillerB:
                    fillerB.popleft()()
                elif c_unlocked[0] and fillerC:
                    fillerC.popleft()()

            def drain_deadline(blk):
                while fillerA and fillerA[0][0] <= blk:
                    fillerA.popleft()[1]()

            # ---------------- attention block ----------------
            def norm_rest_for(o_sb, rs, h, qc):
                def norm_rest():
                    for n in range(2):
                        bcT = psum_x()
                        nc.tensor.matmul(
                            bcT[:],
                            ones_sb[HD : HD + 1, :],
                            rs[HD : HD + 1, n * 512 : (n + 1) * 512],
                            start=True,
                            stop=True,
                            tile_position=(96, 0),
                        )
                        nc.vector.tensor_tensor(
                            out=attn[
                                :,
                                h,
                                qc * 1024 + n * 512 : qc * 1024 + (n + 1) * 512,
                            ],
                            in0=o_sb[0:HD, n * 512 : (n + 1) * 512],
                            in1=bcT[0:HD, :],
                            op=AluOpType.mult,
                        )

                return norm_rest

            def attn_block(qc, h, prev_norm, blk):
                drain_deadline(blk)
                po = psum_po()
                if h == 0:
                    KTh = K3[0:HD, 0, :]
                    QTh = Q3[0:HD, 0, :]
                else:
                    KTh = KT[:, h, :]
                    QTh = QT[:, h, :]
                p_prev = None
                for kv in range(KV_T):
                    s = psum_s()
                    for n in range(2):
                        nc.tensor.matmul(
                            s[:, n * 512 : (n + 1) * 512],
                            KTh[:, kv * 128 : (kv + 1) * 128],
                            QTh[
                                :,
                                qc * 1024 + n * 512 : qc * 1024 + (n + 1) * 512,
                            ],
                            start=True,
                            stop=True,
                        )
                    p = sb.tile([128, 1024], BF16, tag="p", bufs=3, name="p")
                    nc.scalar.activation(
                        p[:], s[:], mybir.ActivationFunctionType.Exp,
                        scale=INV_SQRT_E,
                    )
                    if kv == 7 and prev_norm is not None:
                        prev_norm()
                    drain_one()
                    drain_one()
                    if p_prev is not None:
                        for n in range(2):
                            nc.tensor.matmul(
                                po[0:VW, n * 512 : (n + 1) * 512],
                                V[:, kv - 1, h * VW : (h + 1) * VW],
                                p_prev[:, n * 512 : (n + 1) * 512],
                                start=(kv == 1),
                                stop=False,
                            )
                    p_prev = p
                for n in range(2):
                    nc.tensor.matmul(
                        po[0:VW, n * 512 : (n + 1) * 512],
                        V[:, KV_T - 1, h * VW : (h + 1) * VW],
                        p_prev[:, n * 512 : (n + 1) * 512],
                        start=False,
                        stop=True,
                    )
                o_sb = sb.tile([HD + 1, 1024], F32, tag="osb", bufs=2, name="o_sb")
                nc.vector.tensor_copy(o_sb[:], po[0 : HD + 1, :])
                rs = sb.tile([HD + 1, 1024], BF16, tag="rs", bufs=2, name="rs")
                rf = sb.tile([HD + 1, 1024], F32, tag="rf", bufs=2, name="rf")
                nc.vector.reciprocal_approx_fast(out=rf[:], in_=o_sb[:])
                nc.vector.tensor_copy(rs[HD : HD + 1, :], rf[HD : HD + 1, :])
                return norm_rest_for(o_sb, rs, h, qc)

            # ---------------- preamble ----------------
            for _ in range(55):
                wps = psum_x()
                nc.tensor.matmul(
                    wps[:], warm[:, 0:128], warm[:], start=True, stop=True
                )
            kp_unit(wk_sb, bk_sb, K3, xkv_sb, 0, 0)
            kp_unit(wq_sb, bq_sb, Q3, xq_sb, 0, 0)
            kp_unit(wq_sb, bq_sb, Q3, xq_sb, 0, 1)

            # ---------------- filler queues ----------------
            # order matters: block-0-critical units lead (V(t) before PV
            # uses it, K(h0) chunks before their kv range); repack units
            # are DMA-only and ride along with their producer chunks.
            def K_unit(t3, c):
                return lambda: kp_unit(wk_sb, bk_sb, K3, xkv_sb, t3, c)

            def Q_unit(t3, c):
                return lambda: kp_unit(wq_sb, bq_sb, Q3, xq_sb, t3, c)

            def RK(h, c):
                return lambda: repack_unit(K3, KT, nc.sync, h, c)

            def RQ(h, c):
                return lambda: repack_unit(Q3, QT, nc.scalar, h, c)

            def VU(t):
                return lambda: v_unit(t)

            blk0 = [
                K_unit(0, 1), VU(0), VU(1),
                K_unit(0, 2), VU(2), VU(3),
                K_unit(0, 3), VU(4), VU(5),
                VU(6), VU(7), VU(8), VU(9), VU(10), VU(11),
                VU(12), VU(13), VU(14), VU(15),
            ]
            blk1 = []
            for c in range(4):
                blk1 += [K_unit(1, c), RK(1, c)]
            for c in range(2):
                blk1 += [Q_unit(1, c), RQ(1, c)]
            blk2 = []
            for c in range(4):
                blk2 += [K_unit(2, c), RK(2, c)]
            for c in range(2):
                blk2 += [Q_unit(2, c), RQ(2, c)]
            blk3 = []
            for c in range(4):
                blk3 += [RK(3, c)]
            for c in range(2):
                blk3 += [RQ(3, c)]
            blk4 = []
            for c in range(2, 4):
                blk4 += [Q_unit(0, c), Q_unit(1, c), Q_unit(2, c),
                         RQ(1, c), RQ(2, c), RQ(3, c)]
            fillerA = deque(
                [(1, f) for f in blk0]
                + [(1, f) for f in blk1]
                + [(2, f) for f in blk2]
                + [(3, f) for f in blk3]
                + [(4, f) for f in blk4]
            )
            for qt in range(8):  # outproj for qc0
                fillerB.append(lambda qt=qt: op_unit_x(qt))
            for qt in range(8, QT_T):  # qc1 h0+h1 partials
                fillerC.append(lambda qt=qt: op_h01_unit(qt))

            # ---------------- main loop ----------------
            prev_norm = None
            for qc in range(2):
                for h in range(H_LOCAL):
                    blk = qc * 4 + h
                    if blk == 5:
                        b_unlocked[0] = True
                    if blk == 7:
                        c_unlocked[0] = True
                    prev_norm = attn_block(qc, h, prev_norm, blk)
            # tail: last norm, remaining fillers, outproj qc1
            prev_norm()
            while fillerA:
                fillerA.popleft()[1]()
            b_unlocked[0] = True
            while fillerB:
                fillerB.popleft()()
            c_unlocked[0] = True
            while fillerC:
                fillerC.popleft()()
            for i, qt in enumerate(range(8, QT_T)):
                m = i % 3
                if m == 0:
                    fa = psum_x()
                    fb = psum_x()
                    op_h23_unit(qt, fa[:], fb[:, 0:256])
                elif m == 1:
                    t = psum_s()
                    op_h23_unit(qt, t[:, 0:512], t[:, 512:768])
                else:
                    t = psum_po()
                    op_h23_unit(qt, t[:, 0:512], t[:, 512:768])

    nc.compile()
    return nc


_NC_CACHE = None


def _prep_inputs(x_query, x_kv, Wq, bq, Wk, bk, Wv, bv, Wo, bo):
    import ml_dtypes

    bf16 = ml_dtypes.bfloat16

    def pad_w(W_sl):
        # [384, 768] -> [768, 4, 97] with a zero ones-column, -> [768, 388]
        wp = np.zeros((E, H_LOCAL, VW), dtype=np.float32)
        wt = W_sl.T  # [768, 384]
        for h in range(H_LOCAL):
            wp[:, h, 0:HD] = wt[:, h * HD : (h + 1) * HD]
        return np.ascontiguousarray(wp.reshape(E, DV)).astype(bf16)

    in_maps = []
    for c in range(8):
        b, g = divmod(c, 2)
        sl = slice(g * D, (g + 1) * D)
        bve = np.zeros((DV,), dtype=np.float32)
        for h in range(H_LOCAL):
            bve[h * VW : h * VW + HD] = bv[sl][h * HD : (h + 1) * HD]
            bve[h * VW + HD] = 1.0
        consts = np.zeros((128, 6 + DV), dtype=np.float32)
        consts[:, 0:3] = bk[sl].reshape(3, 128).T
        consts[:, 3:6] = bq[sl].reshape(3, 128).T
        consts[:, 6 : 6 + DV] = bve[None, :]
        in_maps.append(
            {
                "xq_t": np.ascontiguousarray(x_query[b].T).astype(bf16),
                "xkv_t": np.ascontiguousarray(x_kv[b].T).astype(bf16),
                "wq_t": np.ascontiguousarray(Wq[sl, :].T).astype(bf16),
                "wk_t": np.ascontiguousarray(Wk[sl, :].T).astype(bf16),
                "wv_t": pad_w(Wv[sl, :]),
                "wo_t": np.ascontiguousarray(Wo[:, sl].T).astype(bf16),
                "consts_t": consts,
            }
        )
    return in_maps


def kernel(x_query, x_kv, Wq, bq, Wk, bk, Wv, bv, Wo, bo):
    global _NC_CACHE
    x_query = np.asarray(x_query, dtype=np.float32)
    x_kv = np.asarray(x_kv, dtype=np.float32)
    Wq = np.asarray(Wq, dtype=np.float32)
    Wk = np.asarray(Wk, dtype=np.float32)
    Wv = np.asarray(Wv, dtype=np.float32)
    Wo = np.asarray(Wo, dtype=np.float32)
    bq = np.asarray(bq, dtype=np.float32)
    bk = np.asarray(bk, dtype=np.float32)
    bv = np.asarray(bv, dtype=np.float32)
    bo = np.asarray(bo, dtype=np.float32)

    if _NC_CACHE is None:
        _NC_CACHE = build_nc()
    nc = _NC_CACHE

    in_maps = _prep_inputs(x_query, x_kv, Wq, bq, Wk, bk, Wv, bv, Wo, bo)

    trace = bool(int(os.environ.get("KERNEL_TRACE", "0")))
    res = bass_utils.run_bass_kernel_spmd(
        nc, in_maps, core_ids=list(range(8)), trace=trace
    )
    if trace:
        kernel.last_exec_time_ns = res.exec_time_ns
        kernel.last_results = res

    out = np.empty((B, NQ, E), dtype=np.float32)
    for b in range(B):
        out[b] = res.results[2 * b]["out"] + res.results[2 * b + 1]["out"] + bo
    return out


# revision 37
# speedup vs baseline: 1.0266x; 1.0070x over previous
"""Trainium2 Bass kernel for nn_CrossAttention (B=4, N=2048, E=768, H=8).

Sharding: 8 cores = 4 batches x 2 head-groups (4 heads of 96 dims each).
Each core computes its batch's attention for its 4 heads plus the partial
output projection; the host sums the two head-group partials per batch and
adds bo.

Design (v4, ~255us vs the 312us v0 baseline):
  - matmul stationaries are 128 columns where free (K/Q chunks, p tiles)
    for the compiler-automatic Fast Weight Load path; V uses exact 97-wide
    head blocks (96 dims + the softmax-rowsum ones column injected via the
    bias add) -- padding V cost more projection columns than FWL saved.
  - K/Q projections run 128-row packed (3 tiles of 128 dims, not 4 heads
    of 96) and SBUF->SBUF DMAs on the idle software-DGE ring repack heads
    1-3 to partition base 0; head 0 aliases the packed tile directly.
  - the scalar engine runs ONLY exp; the O^T copy and normalization run on
    DVE (single-pass reciprocal_approx_fast over all 97 partitions (~18
    bits, ample under bf16 rounding) -- custom DVE
    ops break at nonzero partition base), and the rowsum reciprocal row is
    broadcast with a 1-row bf16 matmul contracting on partition 96 via
    tile_position=(96,0).  No partition-shift DMA in the norm chain.
  - single software-pipelined emission: projection chunks, repacks and the
    qc0 out-projection are "filler units" drained two per kv-iteration
    into the attention loop so the PE never idles (pstate protection);
    qc1's out-projection is split h0+h1 early / h2+h3 tail, with the
    h0+h1 partials aliased onto xkv's dead SBUF storage.
  - PSUM is exactly 8 banks: tag s 2x[128,1024]f32, po 1x[128,1024],
    x 2x[128,512]; the tail ping-pongs over the then-dead s/po tags.
  - 55 dummy matmuls on a memset scratch tile keep the PE pstate warm
    through the ~20us initial DMA wait (the ramp resets on idle gaps).
  - DMA: ~115GB/s per ring, ~1us fixed cost per DMA, and Act-engine
    dispatches block the engine while its ring is busy -- so the scalar
    ring gets only the two early xq waves, gpsimd (software DGE) carries
    the weights + repacks, sync the rest; qc0 output stores rotate over
    all three rings, tail stores use the two hardware rings only.
"""

import os
import sys
import types
from collections import deque

import numpy as np

# ---------------------------------------------------------------------------
# NTFF profile hook (the agent image's antenv lacks axon_hooks; degrade OK)
# ---------------------------------------------------------------------------
def _install_ntff_hook():
    if "antenv.axon_hooks" in sys.modules:
        return
    try:
        hooks = types.ModuleType("antenv.axon_hooks")
        hooks._hook = None
        hooks.set_axon_ntff_profile_hook = lambda h: setattr(hooks, "_hook", h)
        hooks.get_axon_ntff_profile_hook = lambda: hooks._hook
        sys.modules["antenv.axon_hooks"] = hooks
        import antenv

        antenv.axon_hooks = hooks
        from trn_agent_boot.trn_boot import _ntff_profile_via_ctypes

        so = "/opt/axon/libaxon_pjrt.so"
        if os.path.exists(so):
            hooks.set_axon_ntff_profile_hook(_ntff_profile_via_ctypes(so))
    except Exception:
        pass


_install_ntff_hook()

import concourse.bacc as bacc
import concourse.tile as tile
import concourse.mybir as mybir
from concourse import bass_utils
from concourse.alu_op_type import AluOpType

F32 = mybir.dt.float32
F32R = mybir.dt.float32r
BF16 = mybir.dt.bfloat16

B = 4
NQ = 2048
NKV = 2048
E = 768
H_LOCAL = 4  # heads per core
HD = 96  # head dim
HP = 128  # padded head dim (FWL wants 128-wide stationaries)
D = H_LOCAL * HD  # 384 local proj dim
DP = H_LOCAL * HP  # 512 padded local proj dim
VW = HD + 1  # 97: per-head V block (96 dims + rowsum ones column)
DV = H_LOCAL * VW  # 388
ET = E // 128  # 6 contraction tiles
KV_T = NKV // 128  # 16 kv tiles
QT_T = NQ // 128  # 16 q tiles
INV_SQRT_E = 1.0 / float(np.sqrt(np.float32(E)))


def build_nc():
    nc = bacc.Bacc("TRN2", target_bir_lowering=False, debug=False)

    xq_t = nc.dram_tensor("xq_t", [E, NQ], BF16, kind="ExternalInput")
    xkv_t = nc.dram_tensor("xkv_t", [E, NKV], BF16, kind="ExternalInput")
    wq_t = nc.dram_tensor("wq_t", [E, D], BF16, kind="ExternalInput")
    wk_t = nc.dram_tensor("wk_t", [E, D], BF16, kind="ExternalInput")
    wv_t = nc.dram_tensor("wv_t", [E, DV], BF16, kind="ExternalInput")
    wo_t = nc.dram_tensor("wo_t", [D, E], BF16, kind="ExternalInput")
    consts_t = nc.dram_tensor("consts_t", [128, 6 + DV], F32, kind="ExternalInput")
    out = nc.dram_tensor("out", [NQ, E], F32, kind="ExternalOutput")

    with tile.TileContext(nc) as tc:
        with (
            nc.allow_low_precision(reason="bf16 matmuls and f32r broadcast"),
            tc.tile_pool(name="persist", bufs=1) as persist,
            tc.tile_pool(name="psum", bufs=1, space="PSUM") as pp,
            tc.tile_pool(name="sb", bufs=1) as sb,
        ):
            # ---------------- persistent SBUF tensors ----------------
            KT = persist.tile([HD, H_LOCAL, NKV], BF16)  # K^T per head
            QT = persist.tile([HD, H_LOCAL, NQ], BF16)  # Q^T per head
            # V: [kv-token, kv-tile, head-block(96 v dims + ones + 31 zeros)]
            V = persist.tile([128, KV_T, DV], BF16)
            attn = persist.tile([HD, H_LOCAL, NQ], BF16)  # normalized attn^T
            wo_sb = persist.tile([HD, H_LOCAL, E], BF16)
            wq_sb = persist.tile([128, ET, D], BF16)
            wk_sb = persist.tile([128, ET, D], BF16)
            wv_sb = persist.tile([128, ET, DV], BF16)
            K3 = persist.tile([128, 3, NKV], BF16)  # packed K^T staging
            Q3 = persist.tile([128, 3, NQ], BF16)  # packed Q^T staging
            consts_sb = persist.tile([128, 6 + DV], F32)
            bk_sb = consts_sb[:, 0:3]
            bq_sb = consts_sb[:, 3:6]
            bv_sb = consts_sb[:, 6 : 6 + DV]
            ones_sb = persist.tile([HD + 1, HP], BF16)  # row 96 = [1]*96+[0]*32
            xkv_sb = persist.tile([128, ET, NKV], BF16)
            # qc1 outproj h0+h1 partials alias xkv's storage (dead by then):
            # [128, 6, 2048] bf16 == [128, 6144] f32/2 == [128, 8, 768] f32
            ob01 = xkv_sb[:].bitcast(F32).rearrange("p t n -> p (t n)").rearrange(
                "p (a b) -> p a b", a=8
            )
            xq_sb = persist.tile([128, ET, NQ], BF16)

            # ---------------- input DMAs ----------------
            # Act-engine DMA dispatches block the engine while its ring is
            # busy, so the scalar ring gets ONLY the two early xq waves (ACT
            # must be free for exp by ~15us). gpsimd(software DGE) carries
            # the weights, sync everything else.
            nc.gpsimd.dma_start(
                wq_sb[:], wq_t[:].rearrange("(t p) n -> p t n", p=128)
            )
            nc.gpsimd.dma_start(
                wk_sb[:], wk_t[:].rearrange("(t p) n -> p t n", p=128)
            )
            nc.gpsimd.dma_start(
                wv_sb[:], wv_t[:].rearrange("(t p) n -> p t n", p=128)
            )
            nc.scalar.dma_start(
                xq_sb[:, 0:3, 0:1024],
                xq_t[0:384, 0:1024].rearrange("(t p) n -> p t n", p=128),
            )
            nc.scalar.dma_start(
                xq_sb[:, 3:6, 0:1024],
                xq_t[384:768, 0:1024].rearrange("(t p) n -> p t n", p=128),
            )
            nc.sync.dma_start(
                xkv_sb[:, 0:3, 0:1024],
                xkv_t[0:384, 0:1024].rearrange("(t p) n -> p t n", p=128),
            )
            nc.sync.dma_start(
                xkv_sb[:, 3:6, 0:1024],
                xkv_t[384:768, 0:1024].rearrange("(t p) n -> p t n", p=128),
            )
            nc.sync.dma_start(consts_sb[:], consts_t[:])
            nc.sync.dma_start(
                xkv_sb[:, :, 1024:2048],
                xkv_t[:, 1024:2048].rearrange("(t p) n -> p t n", p=128),
            )
            nc.sync.dma_start(
                xq_sb[:, :, 1024:2048],
                xq_t[:, 1024:2048].rearrange("(t p) n -> p t n", p=128),
            )
            nc.sync.dma_start(
                wo_sb[:], wo_t[:].rearrange("(h p) n -> p h n", p=HD)
            )
            nc.vector.memset(ones_sb[HD : HD + 1, 0:HD], 1.0)
            nc.vector.memset(ones_sb[HD : HD + 1, HD:HP], 0.0)

            # ---------------- PE warm-up ----------------
            # The PE pstate ramp (full speed only after ~3us of continuous
            # busy) resets on every idle gap; the first real matmul waits
            # ~20us for DMA. Keep the array warm through the wait with
            # dummy matmuls on a memset scratch tile (no DMA deps), sized
            # to end roughly when the first input wave lands.
            warm = persist.tile([128, 512], BF16)
            nc.vector.memset(warm[:], 0.0)

            # ---------------- psum slot helpers ----------------
            # tags: s 2x[128,1024], po 1x[128,1024], x 2x[128,512]: 8 banks
            def psum_s():
                return pp.tile([128, 1024], F32, tag="s", bufs=2, name="ps_s")

            def psum_po():
                return pp.tile([128, 1024], F32, tag="po", bufs=1, name="ps_po")

            def psum_x():
                return pp.tile([128, 512], F32, tag="x", bufs=2, name="ps_x")

            # ---------------- projection / outproj unit emitters ----------
            def kp_unit(w_sb, b3, dst3, x_sb, t3, c):
                # dst3[:, t3, c*512:(c+1)*512] = w[t3-block] @ x^T + b
                ps = psum_x()
                for e in range(ET):
                    nc.tensor.matmul(
                        ps[:],
                        w_sb[:, e, t3 * 128 : (t3 + 1) * 128],
                        x_sb[:, e, c * 512 : (c + 1) * 512],
                        start=(e == 0),
                        stop=(e == ET - 1),
                    )
                nc.vector.tensor_scalar_add(
                    out=dst3[:, t3, c * 512 : (c + 1) * 512],
                    in0=ps[:],
                    scalar1=b3[:, t3 : t3 + 1],
                )

            def repack_unit(src3, dst, ring, h, c):
                ring = nc.gpsimd
                # head h dims (96h..96h+95) -> dst[:, h, c-chunk], base 0
                cs = slice(c * 512, (c + 1) * 512)
                lo = h * HD
                t_lo, r_lo = divmod(lo, 128)
                n0 = min(128 - r_lo, HD)
                ring.dma_start(
                    dst[0:n0, h, cs], src3[r_lo : r_lo + n0, t_lo, cs]
                )
                if n0 < HD:
                    ring.dma_start(
                        dst[n0:HD, h, cs],
                        src3[0 : HD - n0, t_lo + 1, cs],
                    )

            def v_unit(t):
                # V[:, t, :] = x_t @ Wv^T + bve  (bve carries the ones column)
                ps = psum_x()
                for e in range(ET):
                    nc.tensor.matmul(
                        ps[:, 0:DV],
                        xkv_sb[:, e, t * 128 : (t + 1) * 128],
                        wv_sb[:, e, :],
                        start=(e == 0),
                        stop=(e == ET - 1),
                    )
                nc.vector.tensor_tensor(
                    out=V[:, t, :],
                    in0=ps[:, 0:DV],
                    in1=bv_sb[:],
                    op=AluOpType.add,
                )

            ob_i = [0]

            def op_unit(qt, fa, fb):
                # out[qt] = attn^T_qt.T @ Wo^T  (accumulate 4 heads)
                for h in range(H_LOCAL):
                    nc.tensor.matmul(
                        fa,
                        attn[:, h, qt * 128 : (qt + 1) * 128],
                        wo_sb[:, h, 0:512],
                        start=(h == 0),
                        stop=(h == H_LOCAL - 1),
                    )
                for h in range(H_LOCAL):
                    nc.tensor.matmul(
                        fb,
                        attn[:, h, qt * 128 : (qt + 1) * 128],
                        wo_sb[:, h, 512:768],
                        start=(h == 0),
                        stop=(h == H_LOCAL - 1),
                    )
                ob = sb.tile([128, E], F32, tag="ob", bufs=2, name="ob")
                nc.vector.tensor_copy(ob[:, 0:512], fa)
                nc.vector.tensor_copy(ob[:, 512:768], fb)
                ring = (nc.sync, nc.scalar, nc.gpsimd)[ob_i[0] % 3]
                ob_i[0] += 1
                ring.dma_start(out[qt * 128 : (qt + 1) * 128, :], ob[:])

            def op_unit_x(qt):
                fa = psum_x()
                fb = psum_x()
                op_unit(qt, fa[:], fb[:, 0:256])

            def op_h01_unit(qt):
                # heads 0-1 accumulation for qc1's out tiles
                fa = psum_x()
                fb = psum_x()
                for h in range(2):
                    nc.tensor.matmul(
                        fa[:],
                        attn[:, h, qt * 128 : (qt + 1) * 128],
                        wo_sb[:, h, 0:512],
                        start=(h == 0),
                        stop=(h == 1),
                    )
                    nc.tensor.matmul(
                        fb[:, 0:256],
                        attn[:, h, qt * 128 : (qt + 1) * 128],
                        wo_sb[:, h, 512:768],
                        start=(h == 0),
                        stop=(h == 1),
                    )
                nc.vector.tensor_copy(ob01[:, qt - 8, 0:512], fa[:])
                nc.vector.tensor_copy(ob01[:, qt - 8, 512:768], fb[:, 0:256])

            def op_h23_unit(qt, fa, fb):
                for h in range(2, 4):
                    nc.tensor.matmul(
                        fa,
                        attn[:, h, qt * 128 : (qt + 1) * 128],
                        wo_sb[:, h, 0:512],
                        start=(h == 2),
                        stop=(h == 3),
                    )
                    nc.tensor.matmul(
                        fb,
                        attn[:, h, qt * 128 : (qt + 1) * 128],
                        wo_sb[:, h, 512:768],
                        start=(h == 2),
                        stop=(h == 3),
                    )
                ob = sb.tile([128, E], F32, tag="ob", bufs=2, name="ob")
                nc.vector.tensor_tensor(
                    out=ob[:, 0:512], in0=fa, in1=ob01[:, qt - 8, 0:512],
                    op=AluOpType.add,
                )
                nc.vector.tensor_tensor(
                    out=ob[:, 512:768], in0=fb, in1=ob01[:, qt - 8, 512:768],
                    op=AluOpType.add,
                )
                ring = (nc.sync, nc.scalar, nc.gpsimd)[ob_i[0] % 3]
                ob_i[0] += 1
                ring.dma_start(out[qt * 128 : (qt + 1) * 128, :], ob[:])

            # ---------------- filler machinery ----------------
            fillerA = deque()  # projection units: (deadline_block, fn)
            fillerB = deque()  # outproj units for qc0 (unlocked at block 5)
            fillerC = deque()  # qc1 outproj h0+h1 partials (unlocked blk 7)
            b_unlocked = [False]
            c_unlocked = [False]

            def drain_one():
                if fillerA:
                    fillerA.popleft()[1]()
                elif b_unlocked[0] and fillerB:
                    fillerB.popleft()()
                elif c_unlocked[0] and fillerC:
                    fillerC.popleft()()

            def drain_deadline(blk):
                while fillerA and fillerA[0][0] <= blk:
                    fillerA.popleft()[1]()

            # ---------------- attention block ----------------
            def norm_rest_for(o_sb, rs, h, qc):
                def norm_rest():
                    for n in range(2):
                        bcT = psum_x()
                        nc.tensor.matmul(
                            bcT[:],
                            ones_sb[HD : HD + 1, :],
                            rs[HD : HD + 1, n * 512 : (n + 1) * 512],
                            start=True,
                            stop=True,
                            tile_position=(96, 0),
                        )
                        nc.vector.tensor_tensor(
                            out=attn[
                                :,
                                h,
                                qc * 1024 + n * 512 : qc * 1024 + (n + 1) * 512,
                            ],
                            in0=o_sb[0:HD, n * 512 : (n + 1) * 512],
                            in1=bcT[0:HD, :],
                            op=AluOpType.mult,
                        )

                return norm_rest

            def attn_block(qc, h, prev_norm, blk):
                drain_deadline(blk)
                po = psum_po()
                if h == 0:
                    KTh = K3[0:HD, 0, :]
                    QTh = Q3[0:HD, 0, :]
                else:
                    KTh = KT[:, h, :]
                    QTh = QT[:, h, :]
                p_prev = None
                for kv in range(KV_T):
                    s = psum_s()
                    for n in range(2):
                        nc.tensor.matmul(
                            s[:, n * 512 : (n + 1) * 512],
                            KTh[:, kv * 128 : (kv + 1) * 128],
                            QTh[
                                :,
                                qc * 1024 + n * 512 : qc * 1024 + (n + 1) * 512,
                            ],
                            start=True,
                            stop=True,
                        )
                    p = sb.tile([128, 1024], BF16, tag="p", bufs=3, name="p")
                    nc.scalar.activation(
                        p[:], s[:], mybir.ActivationFunctionType.Exp,
                        scale=INV_SQRT_E,
                    )
                    if kv == 7 and prev_norm is not None:
                        prev_norm()
                    drain_one()
                    drain_one()
                    if p_prev is not None:
                        for n in range(2):
                            nc.tensor.matmul(
                                po[0:VW, n * 512 : (n + 1) * 512],
                                V[:, kv - 1, h * VW : (h + 1) * VW],
                                p_prev[:, n * 512 : (n + 1) * 512],
                                start=(kv == 1),
                                stop=False,
                            )
                    p_prev = p
                for n in range(2):
                    nc.tensor.matmul(
                        po[0:VW, n * 512 : (n + 1) * 512],
                        V[:, KV_T - 1, h * VW : (h + 1) * VW],
                        p_prev[:, n * 512 : (n + 1) * 512],
                        start=False,
                        stop=True,
                    )
                o_sb = sb.tile([HD + 1, 1024], F32, tag="osb", bufs=2, name="o_sb")
                nc.vector.tensor_copy(o_sb[:], po[0 : HD + 1, :])
                rs = sb.tile([HD + 1, 1024], BF16, tag="rs", bufs=2, name="rs")
                rf = sb.tile([HD + 1, 1024], F32, tag="rf", bufs=2, name="rf")
                nc.vector.reciprocal_approx_fast(out=rf[:], in_=o_sb[:])
                nc.vector.tensor_copy(rs[HD : HD + 1, :], rf[HD : HD + 1, :])
                return norm_rest_for(o_sb, rs, h, qc)

            # ---------------- preamble ----------------
            for _ in range(55):
                wps = psum_x()
                nc.tensor.matmul(
                    wps[:], warm[:, 0:128], warm[:], start=True, stop=True
                )
            kp_unit(wk_sb, bk_sb, K3, xkv_sb, 0, 0)
            kp_unit(wq_sb, bq_sb, Q3, xq_sb, 0, 0)
            kp_unit(wq_sb, bq_sb, Q3, xq_sb, 0, 1)

            # ---------------- filler queues ----------------
            # order matters: block-0-critical units lead (V(t) before PV
            # uses it, K(h0) chunks before their kv range); repack units
            # are DMA-only and ride along with their producer chunks.
            def K_unit(t3, c):
                return lambda: kp_unit(wk_sb, bk_sb, K3, xkv_sb, t3, c)

            def Q_unit(t3, c):
                return lambda: kp_unit(wq_sb, bq_sb, Q3, xq_sb, t3, c)

            def RK(h, c):
                return lambda: repack_unit(K3, KT, nc.sync, h, c)

            def RQ(h, c):
                return lambda: repack_unit(Q3, QT, nc.scalar, h, c)

            def VU(t):
                return lambda: v_unit(t)

            blk0 = [
                K_unit(0, 1), VU(0), VU(1),
                K_unit(0, 2), VU(2), VU(3),
                K_unit(0, 3), VU(4), VU(5),
                VU(6), VU(7), VU(8), VU(9), VU(10), VU(11),
                VU(12), VU(13), VU(14), VU(15),
            ]
            blk1 = []
            for c in range(4):
                blk1 += [K_unit(1, c), RK(1, c)]
            for c in range(2):
                blk1 += [Q_unit(1, c), RQ(1, c)]
            blk2 = []
            for c in range(4):
                blk2 += [K_unit(2, c), RK(2, c)]
            for c in range(2):
                blk2 += [Q_unit(2, c), RQ(2, c)]
            blk3 = []
            for c in range(4):
                blk3 += [RK(3, c)]
            for c in range(2):
                blk3 += [RQ(3, c)]
            blk4 = []
            for c in range(2, 4):
                blk4 += [Q_unit(0, c), Q_unit(1, c), Q_unit(2, c),
                         RQ(1, c), RQ(2, c), RQ(3, c)]
            fillerA = deque(
                [(1, f) for f in blk0]
                + [(1, f) for f in blk1]
                + [(2, f) for f in blk2]
                + [(3, f) for f in blk3]
                + [(4, f) for f in blk4]
            )
            for qt in range(8):  # outproj for qc0
                fillerB.append(lambda qt=qt: op_unit_x(qt))
            for qt in range(8, QT_T):  # qc1 h0+h1 partials
                fillerC.append(lambda qt=qt: op_h01_unit(qt))

            # ---------------- main loop ----------------
            prev_norm = None
            for qc in range(2):
                for h in range(H_LOCAL):
                    blk = qc * 4 + h
                    if blk == 5:
                        b_unlocked[0] = True
                    if blk == 7:
                        c_unlocked[0] = True
                    prev_norm = attn_block(qc, h, prev_norm, blk)
            # tail: last norm, remaining fillers, outproj qc1
            prev_norm()
            while fillerA:
                fillerA.popleft()[1]()
            b_unlocked[0] = True
            while fillerB:
                fillerB.popleft()()
            c_unlocked[0] = True
            while fillerC:
                fillerC.popleft()()
            for i, qt in enumerate(range(8, QT_T)):
                m = i % 3
                if m == 0:
                    fa = psum_x()
                    fb = psum_x()
                    op_h23_unit(qt, fa[:], fb[:, 0:256])
                elif m == 1:
                    t = psum_s()
                    op_h23_unit(qt, t[:, 0:512], t[:, 512:768])
                else:
                    t = psum_po()
                    op_h23_unit(qt, t[:, 0:512], t[:, 512:768])

    nc.compile()
    return nc


_NC_CACHE = None


def _prep_inputs(x_query, x_kv, Wq, bq, Wk, bk, Wv, bv, Wo, bo):
    import ml_dtypes

    bf16 = ml_dtypes.bfloat16

    def pad_w(W_sl):
        # [384, 768] -> [768, 4, 97] with a zero ones-column, -> [768, 388]
        wp = np.zeros((E, H_LOCAL, VW), dtype=np.float32)
        wt = W_sl.T  # [768, 384]
        for h in range(H_LOCAL):
            wp[:, h, 0:HD] = wt[:, h * HD : (h + 1) * HD]
        return np.ascontiguousarray(wp.reshape(E, DV)).astype(bf16)

    in_maps = []
    for c in range(8):
        b, g = divmod(c, 2)
        sl = slice(g * D, (g + 1) * D)
        bve = np.zeros((DV,), dtype=np.float32)
        for h in range(H_LOCAL):
            bve[h * VW : h * VW + HD] = bv[sl][h * HD : (h + 1) * HD]
            bve[h * VW + HD] = 1.0
        consts = np.zeros((128, 6 + DV), dtype=np.float32)
        consts[:, 0:3] = bk[sl].reshape(3, 128).T
        consts[:, 3:6] = bq[sl].reshape(3, 128).T
        consts[:, 6 : 6 + DV] = bve[None, :]
        in_maps.append(
            {
                "xq_t": np.ascontiguousarray(x_query[b].T).astype(bf16),
                "xkv_t": np.ascontiguousarray(x_kv[b].T).astype(bf16),
                "wq_t": np.ascontiguousarray(Wq[sl, :].T).astype(bf16),
                "wk_t": np.ascontiguousarray(Wk[sl, :].T).astype(bf16),
                "wv_t": pad_w(Wv[sl, :]),
                "wo_t": np.ascontiguousarray(Wo[:, sl].T).astype(bf16),
                "consts_t": consts,
            }
        )
    return in_maps


def kernel(x_query, x_kv, Wq, bq, Wk, bk, Wv, bv, Wo, bo):
    global _NC_CACHE
    x_query = np.asarray(x_query, dtype=np.float32)
    x_kv = np.asarray(x_kv, dtype=np.float32)
    Wq = np.asarray(Wq, dtype=np.float32)
    Wk = np.asarray(Wk, dtype=np.float32)
    Wv = np.asarray(Wv, dtype=np.float32)
    Wo = np.asarray(Wo, dtype=np.float32)
    bq = np.asarray(bq, dtype=np.float32)
    bk = np.asarray(bk, dtype=np.float32)
    bv = np.asarray(bv, dtype=np.float32)
    bo = np.asarray(bo, dtype=np.float32)

    if _NC_CACHE is None:
        _NC_CACHE = build_nc()
    nc = _NC_CACHE

    in_maps = _prep_inputs(x_query, x_kv, Wq, bq, Wk, bk, Wv, bv, Wo, bo)

    trace = bool(int(os.environ.get("KERNEL_TRACE", "0")))
    res = bass_utils.run_bass_kernel_spmd(
        nc, in_maps, core_ids=list(range(8)), trace=trace
    )
    if trace:
        kernel.last_exec_time_ns = res.exec_time_ns
        kernel.last_results = res

    out = np.empty((B, NQ, E), dtype=np.float32)
    for b in range(B):
        out[b] = res.results[2 * b]["out"] + res.results[2 * b + 1]["out"] + bo
    return out
